# revision 33
# baseline (speedup 1.0000x reference)
"""Trainium2 Bass kernel for nn_AttentiveModel (B=32,S=128,D=300,P=200,V=30000,C=3).

Data-parallel over batch across 8 NeuronCores (4 batch items per core, all
weights replicated). Trunk compute (highways/projections/compare) runs in
fp16 on the PE; the dist-attention (att2) elementwise is structured so every
bulk DVE op hits a 2x/4x perf mode: those modes require all operands'
INNERMOST access-pattern dim to be stride +-1 / 2-byte, so broadcasts are
placed on middle dims or pre-replicated.

att2[b,j,i] = sum_p 1/(1+|q1[b,i,p]-q2[b,j,p]|), split by p-chunk:

LO chunk (p=0..127 on partitions, free=(b, 8j, 128i) per j-block):
  q2 is replicated only along an 8-wide i-stripe (one DVE seed inst); the
  subtract is a single raw TensorTensor whose operands use different dim
  structures enumerating the same (b,j,i) element order (q1 with j on a
  stride-0 middle dim; q2r8 with a stride-0 i-group dim) -> 2x. Abs is a
  4x tensor_scalar bitwise_and on the int16 view; ScalarE Reciprocal
  (bias=1); partition sums via sliding ones-column PE matmuls into the
  sim PSUM (the first matmul of each 32-row group carries start=True).

HI chunk (p'=0..71 in free, partitions=i, free=(64j, p') per b-half):
  Q2PART (each partition = the flat q2-hi half-row) is built by a flatten
  DMA + 7 partition-doubling DMAs; the 8 chains are spread across the
  sync/gpsimd/scalar DMA queues and consumed LAST on the DVE stream so an
  unfinished chain never blocks in-order DVE progress. Subtract TT 2x
  (q1n-hi with j on a stride-0 middle dim), abs 4x, Reciprocal, then
  in-place fp16 tree folds 72->36->18->9 (TT 2x) + a small fp32
  tensor_reduce; [i,j-half] joins sim via a PE transpose-matmul.

The mul projection + att1 are spliced between att2 windows so the
in-order PE queue interleaves trunk matmuls with the ones-matmuls.
"""

import sys
from contextlib import ExitStack

import numpy as np

for _p in ("/opt/trn_rl_repo",):
    if _p not in sys.path:
        sys.path.insert(0, _p)

import concourse.bass as bass
import concourse.tile as tile
from concourse.bacc import Bacc
from concourse import mybir
from concourse.bass_utils import run_bass_kernel_spmd
from concourse.masks import make_identity

F32 = mybir.dt.float32
BF = mybir.dt.bfloat16
H16 = mybir.dt.float16
I16 = mybir.dt.int16
I32 = mybir.dt.int32
ALU = mybir.AluOpType
ACTF = mybir.ActivationFunctionType
AX = mybir.AxisListType

TRUNK = H16

B, S, D, P, V, C = 32, 128, 300, 200, 30000, 3
NCORES = 8
BL = B // NCORES  # 4 batch items per core
ROWS = BL * S  # 512 per side
ROWS2 = 2 * ROWS  # both sides in one trunk

CH_D = [(0, 128), (128, 128), (256, 44)]  # 300
CH_P = [(0, 128), (128, 72)]  # 200

PLO = 128  # att2 low p-chunk (partition dim)
PHI = 72  # att2 high p-chunk (free dim, layout B)
JB = 8  # j-block size for the LO path
NBLK = S // JB

WEIGHT_NAMES = [
    "hw1_Wh", "hw1_bh", "hw1_Wt", "hw1_bt",
    "hw2_Wh", "hw2_bh", "hw2_Wt", "hw2_bt",
    "mul_W1", "mul_b1", "mul_W2", "mul_b2",
    "dist_W1", "dist_b1", "dist_W2", "dist_b2",
    "cmp_W1", "cmp_b1", "cmp_W2", "cmp_b2",
    "chw1_Wh", "chw1_bh", "chw1_Wt", "chw1_bt",
    "chw2_Wh", "chw2_bh", "chw2_Wt", "chw2_bt",
    "agg_W1", "agg_b1", "agg_W2", "agg_b2",
    "out_W", "out_b",
]

F32_WEIGHTS = {"agg_W1", "agg_W2", "out_W"}


def _chunks(n):
    out = []
    o = 0
    while o < n:
        c = min(128, n - o)
        out.append((o, c))
        o += c
    return out


def act_recip(nc, out, in_, bias=0.0):
    """out = 1/(in_ + bias) in one ScalarE pass (Reciprocal activation)."""
    eng = nc.scalar
    ins_ = [
        eng.lower_ap(in_),
        mybir.ImmediateValue(dtype=mybir.dt.float32, value=bias),
        mybir.ImmediateValue(dtype=mybir.dt.float32, value=1.0),
        mybir.ImmediateValue(dtype=mybir.dt.float32, value=0.0),
    ]
    return eng.add_instruction(
        mybir.InstActivation(
            name=eng.bass.get_next_instruction_name(),
            func=ACTF.Reciprocal,
            ins=ins_,
            outs=[eng.lower_ap(out)],
        )
    )


def build_nc(debug=False):
    nc = Bacc()

    io = {}
    io["x1"] = nc.declare_dram_parameter("x1", [BL, S], I32, isOutput=False)
    io["x2"] = nc.declare_dram_parameter("x2", [BL, S], I32, isOutput=False)
    io["emb"] = nc.declare_dram_parameter("emb", [V, D], F32, isOutput=False)
    shapes = {
        "hw1_Wh": [D, D], "hw1_bh": [D], "hw1_Wt": [D, D], "hw1_bt": [D],
        "hw2_Wh": [D, D], "hw2_bh": [D], "hw2_Wt": [D, D], "hw2_bt": [D],
        "mul_W1": [D, P], "mul_b1": [P], "mul_W2": [P, P], "mul_b2": [P],
        "dist_W1": [D, P], "dist_b1": [P], "dist_W2": [P, P], "dist_b2": [P],
        "cmp_W1": [4 * D, P], "cmp_b1": [P], "cmp_W2": [P, P], "cmp_b2": [P],
        "chw1_Wh": [P, P], "chw1_bh": [P], "chw1_Wt": [P, P], "chw1_bt": [P],
        "chw2_Wh": [P, P], "chw2_bh": [P], "chw2_Wt": [P, P], "chw2_bt": [P],
        "agg_W1": [4 * P, P], "agg_b1": [P], "agg_W2": [P, P], "agg_b2": [P],
        "out_W": [P, C], "out_b": [C],
    }
    for n in WEIGHT_NAMES:
        io[n] = nc.declare_dram_parameter(n, shapes[n], F32, isOutput=False)
    io["yt"] = nc.declare_dram_parameter("yt", [C, BL], F32, isOutput=True)
    if debug:
        io["dbg_sim4"] = nc.declare_dram_parameter("dbg_sim4", [128, 512], F32, isOutput=True)

    with ExitStack() as ctx:
        tc = ctx.enter_context(tile.TileContext(nc))
        _emit(ctx, nc, tc, io, debug=debug)
    nc.finalize()
    return nc


def _emit(ctx, nc, tc, io, debug=False):
    wpool = ctx.enter_context(tc.tile_pool(name="wpool", bufs=1))
    wstage = ctx.enter_context(tc.tile_pool(name="wstage", bufs=2))
    const = ctx.enter_context(tc.tile_pool(name="const", bufs=1))
    persist = ctx.enter_context(tc.tile_pool(name="persist", bufs=1))
    work = ctx.enter_context(tc.tile_pool(name="work", bufs=1))
    small = ctx.enter_context(tc.tile_pool(name="small", bufs=3))

    pp_mm = ctx.enter_context(tc.tile_pool(name="pp_mm", bufs=3, space="PSUM"))
    pp_sim = ctx.enter_context(tc.tile_pool(name="pp_sim", bufs=1, space="PSUM"))
    pp_tr = ctx.enter_context(tc.tile_pool(name="pp_tr", bufs=2, space="PSUM"))
    pp_sm = ctx.enter_context(tc.tile_pool(name="pp_sm", bufs=2, space="PSUM"))

    # ---------------- constants ----------------
    identf = const.tile([128, 128], F32, tag="identf", name="identf")
    make_identity(nc, identf[:, :])
    identr = const.tile([128, 128], TRUNK, tag="identr", name="identr")
    nc.vector.tensor_scalar_add(out=identr[:, :], in0=identf[:, :], scalar1=0.0)
    identb = const.tile([128, 128], H16, tag="identb", name="identb")
    nc.vector.tensor_scalar_add(out=identb[:, :], in0=identf[:, :], scalar1=0.0)

    # sliding ones-column buffer: Z[:, 32] == 1 so Z[:, 32-r:64-r] has its
    # ones in column r; Z_slice.T @ U deposits column-sums of U into row r.
    zbuf = const.tile([128, 64], H16, tag="zbuf", name="zbuf")
    nc.vector.memset(zbuf[:, :], 0.0)
    nc.vector.memset(zbuf[:, 32:33], 1.0)

    # ---------------- weights: casting DMAs ----------------
    SPECIAL_KCH = {
        "cmp_W1": [(s * D + o, c) for s in range(4) for (o, c) in CH_D],
        "agg_W1": [(s * P + o, c) for s in range(4) for (o, c) in CH_P],
    }

    def load_w(name):
        h = io[name]
        K, M = h.shape
        dt = F32 if name in F32_WEIGHTS else H16
        tiles = []
        for i, (o, c) in enumerate(SPECIAL_KCH.get(name, _chunks(K))):
            t = wpool.tile([c, M], dt, tag=f"w_{name}_{i}", name=f"w_{name}_{i}")
            if dt == F32:
                nc.sync.dma_start(out=t[:, :], in_=h[o:o + c, :])
            else:
                stg = wstage.tile([128, M], F32, tag="wstg", name=f"wstg_{name}_{i}")
                nc.sync.dma_start(out=stg[:c, :], in_=h[o:o + c, :])
                nc.vector.tensor_scalar_add(out=t[:, :], in0=stg[:c, :], scalar1=0.0)
            tiles.append(t)
        return tiles

    def load_b(name):
        h = io[name]
        (M,) = h.shape
        tiles = []
        for i, (o, c) in enumerate(_chunks(M)):
            t = wpool.tile([c, 1], F32, tag=f"b_{name}_{i}", name=f"b_{name}_{i}")
            nc.sync.dma_start(out=t[:, :], in_=h[o:o + c])
            tiles.append(t)
        return tiles

    # ---------------- index DMAs + gathers (overlap weight DMAs) ----------
    pre2 = ctx.enter_context(ExitStack())
    g2pool = pre2.enter_context(tc.tile_pool(name="g2pool", bufs=1))
    pre1 = ctx.enter_context(ExitStack())
    gpool = pre1.enter_context(tc.tile_pool(name="gpool", bufs=1))
    e_n = {}
    for side, xh in (("1", io["x1"]), ("2", io["x2"])):
        for b in range(BL):
            idx = gpool.tile([128, 1], I32, tag=f"idx{side}_{b}", name=f"idx{side}_{b}")
            nc.sync.dma_start(out=idx[:, :], in_=xh[b, :])
            e = gpool.tile([128, D], H16, tag=f"e{side}_{b}", name=f"e{side}_{b}")
            nc.gpsimd.indirect_dma_start(
                out=e[:, :], out_offset=None, in_=io["emb"][:, :],
                in_offset=bass.IndirectOffsetOnAxis(ap=idx[:, :1], axis=0),
            )
            e_n[(side, b)] = e

    W = {}
    for n in WEIGHT_NAMES:
        W[n] = load_b(n) if n.endswith(("bh", "bt", "b1", "b2", "_b")) else load_w(n)

    # ---------------- helpers ----------------
    def mm_apply(w_tiles, b_tiles, rhs_tiles, n_free, func, out_tiles,
                 krange=None, mrange=None):
        """out = func(W.T @ rhs + b), transposed layout, 512-col PSUM chunks."""
        M = w_tiles[0].shape[1]
        mch = _chunks(M)
        ks = list(range(len(w_tiles))) if krange is None else krange
        m_iter = ([(i, i) for i in range(len(mch))] if mrange is None
                  else list(enumerate(mrange)))
        for oi, mi in m_iter:
            mo, mc = mch[mi]
            for fo in range(0, n_free, 512):
                fc = min(512, n_free - fo)
                ps = pp_mm.tile([128, 512], F32, tag="mmout", name="mmout")
                for idx, ki in enumerate(ks):
                    kc = w_tiles[ki].shape[0]
                    nc.tensor.matmul(
                        out=ps[:mc, :fc],
                        lhsT=w_tiles[ki][:kc, mo:mo + mc],
                        rhs=rhs_tiles[ki][:kc, fo:fo + fc],
                        start=(idx == 0),
                        stop=(idx == len(ks) - 1),
                    )
                nc.scalar.activation(
                    out=out_tiles[oi][:mc, fo:fo + fc],
                    in_=ps[:mc, :fc],
                    func=func, bias=b_tiles[mi][:mc, :], scale=1.0,
                )

    def highway(xt_tiles, wh, bh, wt, bt, feat, out_tiles):
        """out = x + t*(h-x), trunk layout, chunk-at-a-time (h reused as tmp)."""
        ch = _chunks(feat)
        for mi, (mo, mc) in enumerate(ch):
            h = work.tile([128, ROWS2], TRUNK, tag=f"hw_h{mi % 2}", name="hw_h")
            t = work.tile([128, ROWS2], TRUNK, tag=f"hw_t{mi % 2}", name="hw_t")
            mm_apply(wh, bh, xt_tiles, ROWS2, ACTF.Relu, [h], mrange=[mi])
            mm_apply(wt, bt, xt_tiles, ROWS2, ACTF.Sigmoid, [t], mrange=[mi])
            x_sl = xt_tiles[mi][:mc, :]
            nc.vector.tensor_tensor(out=h[:mc, :], in0=h[:mc, :], in1=x_sl,
                                    op=ALU.subtract)
            nc.vector.tensor_tensor(out=h[:mc, :], in0=h[:mc, :], in1=t[:mc, :],
                                    op=ALU.mult)
            nc.vector.tensor_tensor(out=out_tiles[mi][:mc, :], in0=h[:mc, :],
                                    in1=x_sl, op=ALU.add)

    # ---------------- embed: transpose into trunk ----------------
    eT = [g2pool.tile([128, ROWS2], TRUNK, tag=f"eT_{i}", name=f"eT_{i}")
          for i in range(3)]
    for ki, (ko, kc) in enumerate(CH_D):
        for side in ("1", "2"):
            ps = pp_tr.tile([128, 512], H16, tag="trpackb", name="trpack")
            for b in range(BL):
                nc.tensor.transpose(
                    out=ps[:kc, b * S:(b + 1) * S],
                    in_=e_n[(side, b)][:, ko:ko + kc],
                    identity=identb[:128, :128],
                )
            so = (0 if side == "1" else ROWS)
            nc.scalar.activation(out=eT[ki][:kc, so:so + ROWS], in_=ps[:kc, :ROWS],
                                 func=ACTF.Copy)
    pre1.close()  # frees index + gather tiles

    # ---------------- highway stack (trunk: both sides at once) -------------
    h1 = [g2pool.tile([128, ROWS2], TRUNK, tag=f"hwy1_{i}", name=f"hwy1_{i}")
          for i in range(3)]
    highway(eT, W["hw1_Wh"], W["hw1_bh"], W["hw1_Wt"], W["hw1_bt"], D, h1)
    eTh = [persist.tile([128, ROWS2], TRUNK, tag=f"eTh_{i}", name=f"eTh_{i}")
           for i in range(3)]
    highway(h1, W["hw2_Wh"], W["hw2_bh"], W["hw2_Wt"], W["hw2_bt"], D, eTh)
    pre2.close()  # frees eT, h1

    # ---------------- projections (shared weights, trunk) ----------------
    def proj(prefix, pool):
        z1 = [work.tile([128, ROWS2], TRUNK, tag=f"z1_{i}", name=f"z1_{i}") for i in range(2)]
        mm_apply(W[f"{prefix}_W1"], W[f"{prefix}_b1"], eTh, ROWS2, ACTF.Relu, z1)
        out = [pool.tile([128, ROWS2], TRUNK, tag=f"{prefix}T_{i}", name=f"{prefix}T_{i}")
               for i in range(2)]
        mm_apply(W[f"{prefix}_W2"], W[f"{prefix}_b2"], z1, ROWS2, ACTF.Relu, out)
        return out

    # dist first so the att2 elementwise can start while the PE continues
    # with the mul projection; hi chunk (m=1) first for earlier hi-prep
    qT = proj("dist", persist)
    # qT[0]: p 0..127 [128, (side,b,t)]; qT[1][:72]: p 128..199

    # ---- att2 prep: hi-chunk transposes to normal layout, per b ----
    # (pools opened here, after the gather/highway scratch is freed)
    q2ppool = ctx.enter_context(tc.tile_pool(name="q2ppool", bufs=8))
    uhipool = ctx.enter_context(tc.tile_pool(name="uhipool", bufs=2))
    q2rpool = ctx.enter_context(tc.tile_pool(name="q2rpool", bufs=2))
    ulopool = ctx.enter_context(tc.tile_pool(name="ulopool", bufs=3))
    shpool = ctx.enter_context(tc.tile_pool(name="shpool", bufs=2))
    q1n_hi, q2part, q2nh = {}, {}, {}
    for b in range(BL):
        ps = pp_tr.tile([128, 512], H16, tag="trpackb", name=f"hitr_{b}")
        nc.tensor.transpose(out=ps[:128, 0:PHI],
                            in_=qT[1][:PHI, b * S:(b + 1) * S],
                            identity=identr[:PHI, :PHI])
        nc.tensor.transpose(out=ps[:128, 128:128 + PHI],
                            in_=qT[1][:PHI, ROWS + b * S:ROWS + (b + 1) * S],
                            identity=identr[:PHI, :PHI])
        t1 = persist.tile([128, PHI], H16, tag=f"q1nh_{b}", name=f"q1nh_{b}")
        nc.vector.tensor_scalar_add(out=t1[:, :], in0=ps[:128, 0:PHI], scalar1=0.0)
        q1n_hi[b] = t1
        t2 = persist.tile([128, PHI], H16, tag=f"q2nh_{b}", name=f"q2nh_{b}")
        nc.vector.tensor_scalar_add(out=t2[:, :], in0=ps[:128, 128:128 + PHI],
                                    scalar1=0.0)
        q2nh[b] = t2

    # Q2PART[b,h]: [128 i, (64 j, p')], every partition = the flat q2n_hi
    # half-row; flatten seed + 7 doubling rounds. Emitted ROUND-MAJOR
    # across all 8 chains: each DMA's dependency was issued 8 slots
    # earlier, so the in-order sync queue never blocks on a waiting head.
    SHH = 64
    for b in range(BL):
        for h in range(2):
            q2part[(b, h)] = q2ppool.tile([128, SHH * PHI], H16, tag="q2part",
                                          name=f"q2p_{b}_{h}")
    for b in range(BL):
        for h in range(2):
            qp = q2part[(b, h)]
            qeng = (nc.sync, nc.sync, nc.gpsimd, nc.scalar)[b]
            qeng.dma_start(out=qp[0:1, :],
                           in_=q2nh[b][h * SHH:(h + 1) * SHH, :])
            n = 1
            while n < 128:
                qeng.dma_start(out=qp[n:2 * n, :], in_=qp[0:n, :])
                n *= 2

    # normal-layout post-highway embeddings (lhsT for the beta/alpha
    # matmuls); PE runs these during its att2 idle windows.
    ehw_n = {}
    for side in ("1", "2"):
        so = (0 if side == "1" else ROWS)
        for b in range(BL):
            ps = pp_tr.tile([128, 512], H16, tag="trpackb", name="trpackr")
            for ki, (ko, kc) in enumerate(CH_D):
                nc.tensor.transpose(
                    out=ps[:128, ko:ko + kc],
                    in_=eTh[ki][:kc, so + b * S:so + (b + 1) * S],
                    identity=identr[:kc, :kc],
                )
            t = persist.tile([128, D], H16, tag=f"ehwn{side}_{b}", name=f"ehwn{side}_{b}")
            nc.scalar.activation(out=t[:, :], in_=ps[:, :D], func=ACTF.Copy)
            ehw_n[(side, b)] = t

    # sim4 PSUM accumulates (in PE order): lo ones-matmuls + hi transposes
    # (emitted in the att2 loop, first one carries start=True) + att1
    # (emitted between windows once the mul projection is done).
    sim4 = pp_sim.tile([128, 512], F32, tag="sim4", name="sim4")
    sim4_group_started = [False] * 4

    # ---------------- att2 ----------------
    q1lo = qT[0][:PLO, 0:ROWS]  # [p, (b,i)]
    q2lo = qT[0][:PLO, ROWS:ROWS2]  # [p, (b,j)]

    W8 = 8  # replicated q2 stripe width for the LO subtract

    def raw_tt(out, in0, in1, op):
        """TensorTensor with shape-mismatched APs (same element order)."""
        eng = nc.vector
        return eng.add_instruction(
            mybir.InstTensorTensor(
                name=eng.bass.get_next_instruction_name(),
                op=op,
                ins=[eng.lower_ap(in0), eng.lower_ap(in1)],
                outs=[eng.lower_ap(out)],
            )
        )

    def lo_block(jb, abs_on_scalar=False):
        """LO p-chunk, j-block jb: u[p, (b, 8j, 128i)] -> ones-matmuls."""
        ncols = BL * JB * S  # 4096
        # q2r8: q2lo replicated along an i-stripe of width 8 (one 1x DVE
        # seed, 256 cols), laid out (b, j, i8). The subtract reads it with
        # a stride-0 MIDDLE dim (i-group): all inner dims contiguous -> 2x.
        q2r = q2rpool.tile([128, BL * JB * W8], H16, tag="q2r", name=f"q2r_{jb}")
        seed_in = bass.AP(
            tensor=q2lo.tensor, offset=q2lo.offset + jb * JB,
            ap=[q2lo.ap[0], [S, BL], [1, JB], [0, W8]])
        seed_out = bass.AP(
            tensor=q2r.tensor, offset=q2r.offset,
            ap=[q2r.ap[0], [JB * W8, BL], [W8, JB], [1, W8]])
        nc.vector.tensor_scalar_add(out=seed_out, in0=seed_in, scalar1=0.0)
        # subtract (TT 2x), ONE inst per block: both APs enumerate
        # (b, j, i) in the same order but with different dim structures
        # (bass's free-shape assert is bypassed; the ISA only streams).
        u = ulopool.tile([128, ncols], H16, tag="ulo", name=f"ulo_{jb}")
        in0 = bass.AP(tensor=q1lo.tensor, offset=q1lo.offset,
                      ap=[q1lo.ap[0], [S, BL], [0, JB], [1, S]])
        in1 = bass.AP(tensor=q2r.tensor, offset=q2r.offset,
                      ap=[q2r.ap[0], [W8, BL * JB], [0, S // W8], [1, W8]])
        uo = bass.AP(tensor=u.tensor, offset=u.offset,
                     ap=[u.ap[0], [1, ncols]])
        raw_tt(uo, in0, in1, ALU.subtract)
        if abs_on_scalar:
            # |u| then 1/(1+|u|), both on ScalarE (DVE relief)
            nc.scalar.activation(out=u[:, :], in_=u[:, :], func=ACTF.Abs)
        else:
            ui = u[:128, :].bitcast(I16)
            nc.vector.tensor_scalar(out=ui, in0=ui, scalar1=0x7FFF, scalar2=None,
                                    op0=ALU.bitwise_and)
        act_recip(nc, u[:, :], u[:, :], bias=1.0)
        # partition sums into sim4 rows via sliding ones-columns
        for jj in range(JB):
            j = jb * JB + jj
            g, rr = j // 32, j % 32
            rbase = u[:128, jj * S:jj * S + S]
            rhs = bass.AP(tensor=rbase.tensor, offset=rbase.offset,
                          ap=[rbase.ap[0], [JB * S, BL], [1, S]])
            nc.tensor.matmul(
                out=sim4[32 * g:32 * g + 32, :],
                lhsT=zbuf[:128, 32 - rr:64 - rr],
                rhs=rhs,
                start=(not sim4_group_started[g]), stop=False,
                skip_group_check=True,
                tile_position=(0, 32 * g),
            )
            sim4_group_started[g] = True

    SH = S // 2  # hi-path j-half size
    hi_done = []

    def hi_half(b, h):
        """HI p-chunk, batch b, j-half h: u[i, (64j, p')] -> tree -> transpose."""
        ncols = SH * PHI  # 4608
        qp = q2part[(b, h)]
        u = uhipool.tile([128, ncols], H16, tag="uhi", name=f"uhi_{b}_{h}")
        in0 = bass.AP(tensor=q1n_hi[b].tensor, offset=q1n_hi[b].offset,
                      ap=[q1n_hi[b].ap[0], [0, SH], [1, PHI]])
        u3 = u[:128, :].rearrange("i (j p) -> i j p", j=SH)
        qp3 = qp[:128, :].rearrange("i (j p) -> i j p", j=SH)
        nc.vector.tensor_tensor(out=u3, in0=in0, in1=qp3, op=ALU.subtract)
        ui = u[:128, :].bitcast(I16)
        nc.vector.tensor_scalar(out=ui, in0=ui, scalar1=0x7FFF, scalar2=None,
                                op0=ALU.bitwise_and)
        act_recip(nc, u[:, :], u[:, :], bias=1.0)
        # in-place fp16 tree folds: 72 -> 36 -> 18 (TT 2x)
        nc.vector.tensor_tensor(out=u3[:, :, 0:36], in0=u3[:, :, 0:36],
                                in1=u3[:, :, 36:72], op=ALU.add)
        nc.vector.tensor_tensor(out=u3[:, :, 0:18], in0=u3[:, :, 0:18],
                                in1=u3[:, :, 18:36], op=ALU.add)
        nc.vector.tensor_tensor(out=u3[:, :, 0:9], in0=u3[:, :, 0:9],
                                in1=u3[:, :, 9:18], op=ALU.add)
        sh = shpool.tile([128, SH], F32, tag="simhi", name=f"simhi_{b}_{h}")
        nc.vector.tensor_reduce(out=sh[:, :], in_=u3[:, :, 0:9], axis=AX.X,
                                op=ALU.add)
        shh = persist.tile([128, SH], H16, tag=f"simhih_{b}_{h}",
                           name=f"simhih_{b}_{h}")
        nc.vector.tensor_scalar_add(out=shh[:, :], in0=sh[:, :], scalar1=0.0)
        hi_done.append((b, h, shh))

    # interleave: att2 windows with the mul-projection pieces + att1 (PE
    # executes its queue in order, so trunk matmuls are spliced between
    # the windows' ones-matmuls).
    z1m = [work.tile([128, ROWS2], TRUNK, tag=f"z1_{i}", name=f"z1m_{i}")
           for i in range(2)]
    pT = [persist.tile([128, ROWS2], TRUNK, tag=f"mulT_{i}", name=f"mulT_{i}")
          for i in range(2)]

    def mul_piece(k):
        if k < 2:
            mm_apply(W["mul_W1"], W["mul_b1"], eTh, ROWS2, ACTF.Relu,
                     [z1m[k]], mrange=[k])
        else:
            mm_apply(W["mul_W2"], W["mul_b2"], z1m, ROWS2, ACTF.Relu,
                     [pT[k - 2]], mrange=[k - 2])

    ABS_SCALAR_BLOCKS = {3, 7, 11, 15}
    # all lo blocks first: they have no DMA dependency, so the in-order
    # DVE stream never stalls on a Q2PART chain while lo work remains
    for b in range(BL):
        for jb in range(4 * b, 4 * b + 4):
            lo_block(jb, abs_on_scalar=(jb in ABS_SCALAR_BLOCKS))
        mul_piece(b)
    # hi halves last, in chain-completion order
    for b in range(BL):
        for h in range(2):
            hi_half(b, h)
    # att1
    for bb in range(BL):
        for ki, (ko, kc) in enumerate(CH_P):
            nc.tensor.matmul(
                out=sim4[:, bb * S:(bb + 1) * S],
                lhsT=pT[ki][:kc, ROWS + bb * S:ROWS + (bb + 1) * S],
                rhs=pT[ki][:kc, bb * S:(bb + 1) * S],
                start=False, stop=False, skip_group_check=True,
            )
    # deferred hi transpose-accumulates [i, j-half] -> sim4
    for (bb, h, shh) in hi_done:
        nc.tensor.matmul(
            out=sim4[h * SH:(h + 1) * SH, bb * S:(bb + 1) * S],
            lhsT=shh[:, :], rhs=identb[:, :],
            start=False, stop=False, skip_group_check=True,
            tile_position=(0, h * SH),
        )

    # close the sim4 accumulation group with a dummy stopping matmul on a
    # zero rhs? Not needed: readers below read PSUM after all writers.

    if debug and "dbg_sim4" in io:
        t = small.tile([128, 512], F32, tag="dbgps", name="dbg_sim4_t")
        nc.scalar.activation(out=t[:, :], in_=sim4[:, :], func=ACTF.Copy)
        nc.gpsimd.dma_start(out=io["dbg_sim4"][:, :], in_=t[:, :])

    # ---------------- softmax + beta/alpha + compare part 1 ----------------
    def softmax_p(src_psum):
        """softmax over rows of src [128,128]; returns transposed probs fp16."""
        mx = small.tile([128, 1], F32, tag="sm_mx", name="sm_mx")
        nc.vector.tensor_reduce(out=mx[:, :], in_=src_psum, axis=AX.X,
                                op=ALU.max, negate=True)
        esb = small.tile([128, S], H16, tag="sm_e", name="sm_e")
        zs = small.tile([128, 1], F32, tag="sm_z", name="sm_z")
        nc.scalar.activation(out=esb[:, :], in_=src_psum, func=ACTF.Exp,
                             bias=mx[:, :], scale=1.0, accum_out=zs[:, :])
        rz = small.tile([128, 1], F32, tag="sm_rz", name="sm_rz")
        nc.vector.reciprocal(out=rz[:, :], in_=zs[:, :])
        pr = small.tile([128, S], H16, tag="sm_p", name="sm_p")
        nc.vector.tensor_scalar(out=pr[:, :], in0=esb[:, :], scalar1=rz[:, :],
                                scalar2=None, op0=ALU.mult)
        pt_ps = pp_tr.tile([128, 512], H16, tag="trpackb", name="trpackb")
        nc.tensor.transpose(out=pt_ps[:S, :S], in_=pr[:, :], identity=identb[:, :])
        pt = small.tile([128, S], H16, tag="sm_pt", name="sm_pt")
        nc.vector.tensor_scalar_add(out=pt[:, :], in0=pt_ps[:S, :S], scalar1=0.0)
        return pt

    betaT = {s: [persist.tile([128, 512], H16, tag=f"betaT{s}_{i}", name=f"betaT{s}_{i}")
                 for i in range(3)] for s in ("1", "2")}

    for b in range(BL):
        bs4 = sim4[:, b * S:(b + 1) * S]
        simT_sb = small.tile([128, S], F32, tag="simT_sb", name="simT_sb")
        nc.vector.tensor_scalar_add(out=simT_sb[:, :], in0=bs4, scalar1=0.0)
        sim_ps = pp_sm.tile([128, S], F32, tag="btps", name="simtr")
        nc.tensor.transpose(out=sim_ps[:S, :S], in_=simT_sb[:, :],
                            identity=identf[:, :])
        ptA = softmax_p(bs4)  # alpha probs^T [i, j]
        ptB = softmax_p(sim_ps[:S, :S])  # beta probs^T [j, i]

        for side, pt, eln in (("2", ptA, "1"), ("1", ptB, "2")):
            for ki, (ko, kc) in enumerate(CH_D):
                bt_ps = pp_sm.tile([128, S], F32, tag="btps", name="btps")
                nc.tensor.matmul(
                    out=bt_ps[:kc, :], lhsT=ehw_n[(eln, b)][:, ko:ko + kc],
                    rhs=pt[:, :], start=True, stop=True,
                )
                nc.vector.tensor_scalar_add(
                    out=betaT[side][ki][:kc, b * S:(b + 1) * S],
                    in0=bt_ps[:kc, :], scalar1=0.0)

    # cat + compare matmul, per side over 512-col trunk halves.
    cmp1 = [persist.tile([128, ROWS2], H16, tag=f"cmp1_{i}", name=f"cmp1_{i}")
            for i in range(2)]
    for side in ("1", "2"):
        so = (0 if side == "1" else ROWS)
        ps2 = [pp_mm.tile([128, 512], F32, tag="mmout", name=f"cmp1ps{mi}")
               for mi in range(2)]
        for sel in range(4):  # e, beta, e-beta, e*beta
            for ki, (ko, kc) in enumerate(CH_D):
                e_sl = eTh[ki][:kc, so:so + ROWS]
                b_sl = betaT[side][ki][:kc, :]
                if sel == 0:
                    rhs = e_sl  # pure-e chunk: use eTh directly, no copy
                elif sel == 1:
                    rhs = b_sl
                else:
                    cat = small.tile([128, 512], H16, tag="cat", name="cat")
                    nc.vector.tensor_tensor(
                        out=cat[:kc, :], in0=e_sl, in1=b_sl,
                        op=(ALU.subtract if sel == 2 else ALU.mult))
                    rhs = cat[:kc, :]
                idx = sel * 3 + ki
                for mi, (mo, mc) in enumerate(CH_P):
                    nc.tensor.matmul(
                        out=ps2[mi][:mc, :],
                        lhsT=W["cmp_W1"][idx][:, mo:mo + mc],
                        rhs=rhs,
                        start=(idx == 0), stop=(idx == 11),
                        skip_group_check=True,
                    )
        for mi, (mo, mc) in enumerate(CH_P):
            nc.scalar.activation(
                out=cmp1[mi][:mc, so:so + ROWS], in_=ps2[mi][:mc, :],
                func=ACTF.Relu,
                bias=W["cmp_b1"][mi][:mc, :], scale=1.0,
            )

    # ---------------- compare part 2 + compare highway (trunk) --------------
    v0 = [work.tile([128, ROWS2], H16, tag=f"z1_{i}", name=f"v0_{i}") for i in range(2)]
    mm_apply(W["cmp_W2"], W["cmp_b2"], cmp1, ROWS2, ACTF.Relu, v0)
    v1 = [work.tile([128, ROWS2], H16, tag=f"v1_{i}", name=f"v1_{i}") for i in range(2)]
    highway(v0, W["chw1_Wh"], W["chw1_bh"], W["chw1_Wt"], W["chw1_bt"], P, v1)
    vT = [persist.tile([128, ROWS2], H16, tag=f"vT_{i}", name=f"vT_{i}")
          for i in range(2)]
    highway(v1, W["chw2_Wh"], W["chw2_bh"], W["chw2_Wt"], W["chw2_bt"], P, vT)

    # ---------------- aggregate (fp32) ----------------
    stats = []
    for sect, (side, op) in enumerate(
            (("1", ALU.max), ("2", ALU.max), ("1", ALU.add), ("2", ALU.add))):
        so = (0 if side == "1" else ROWS)
        st = [persist.tile([128, BL], F32, tag=f"st{sect}_{i}", name=f"st{sect}_{i}")
              for i in range(2)]
        for ki, (ko, kc) in enumerate(CH_P):
            seg = vT[ki][:kc, so:so + ROWS].rearrange("p (b t) -> p b t", b=BL)
            nc.vector.tensor_reduce(
                out=st[ki][:kc, :BL], in_=seg, axis=AX.X, op=op,
            )
        stats.append(st)

    agg_rhs = [stats[s][ki] for s in range(4) for ki in range(2)]
    y1 = [persist.tile([128, BL], F32, tag=f"y1_{i}", name=f"y1_{i}") for i in range(2)]
    mm_apply(W["agg_W1"], W["agg_b1"], agg_rhs, BL, ACTF.Relu, y1)
    y2 = [persist.tile([128, BL], F32, tag=f"y2_{i}", name=f"y2_{i}") for i in range(2)]
    mm_apply(W["agg_W2"], W["agg_b2"], y1, BL, ACTF.Relu, y2)

    yt_ps = pp_sm.tile([128, S], F32, tag="btps", name="btps")
    for ki, (ko, kc) in enumerate(CH_P):
        nc.tensor.matmul(
            out=yt_ps[:C, :BL], lhsT=W["out_W"][ki][:kc, :],
            rhs=y2[ki][:kc, :], start=(ki == 0), stop=(ki == 1),
        )
    yt_sb = persist.tile([C, BL], F32, tag="yt_sb", name="yt_sb")
    nc.scalar.activation(out=yt_sb[:, :], in_=yt_ps[:C, :BL], func=ACTF.Identity,
                         bias=W["out_b"][0][:C, :], scale=1.0)
    nc.sync.dma_start(out=io["yt"][:, :], in_=yt_sb[:, :])


_NC_CACHE = {}


def _get_nc():
    if "nc" not in _NC_CACHE:
        _NC_CACHE["nc"] = build_nc()
    return _NC_CACHE["nc"]


def make_in_maps(inputs):
    """Shard full inputs into 8 per-core input maps."""
    x1 = np.ascontiguousarray(np.asarray(inputs["x1"]).astype(np.int32))
    x2 = np.ascontiguousarray(np.asarray(inputs["x2"]).astype(np.int32))
    shared = {}
    for n in WEIGHT_NAMES + ["emb"]:
        shared[n] = np.ascontiguousarray(np.asarray(inputs[n]).astype(np.float32))
    in_maps = []
    for c in range(NCORES):
        m = dict(shared)
        m["x1"] = x1[c * BL:(c + 1) * BL]
        m["x2"] = x2[c * BL:(c + 1) * BL]
        in_maps.append(m)
    return in_maps


def kernel(**inputs):
    nc = _get_nc()
    in_maps = make_in_maps(inputs)
    res = run_bass_kernel_spmd(nc, in_maps, core_ids=list(range(NCORES)))
    return np.concatenate([np.asarray(r["yt"]).T for r in res.results], axis=0)


if __name__ == "__main__":
    nc = build_nc()
    print("built ok")


# revision 34
# speedup vs baseline: 1.0908x; 1.0908x over previous
"""Trainium2 Bass kernel for nn_AttentiveModel (B=32,S=128,D=300,P=200,V=30000,C=3).

Data-parallel over batch across 8 NeuronCores (4 batch items per core, all
weights replicated). Trunk compute (highways/projections/compare) runs in
fp16 on the PE; the dist-attention (att2) elementwise is structured so every
bulk DVE op hits a 2x/4x perf mode: those modes require all operands'
INNERMOST access-pattern dim to be stride +-1 / 2-byte, so broadcasts are
placed on middle dims or pre-replicated.

att2[b,j,i] = sum_p 1/(1+|q1[b,i,p]-q2[b,j,p]|), split by p-chunk:

LO chunk (p=0..127 on partitions, free=(b, 8j, 128i) per j-block):
  q2 is replicated only along an 8-wide i-stripe (one DVE seed inst); the
  subtract is a single raw TensorTensor whose operands use different dim
  structures enumerating the same (b,j,i) element order (q1 with j on a
  stride-0 middle dim; q2r8 with a stride-0 i-group dim) -> 2x. Abs is a
  4x tensor_scalar bitwise_and on the int16 view; ScalarE Reciprocal
  (bias=1); partition sums via sliding ones-column PE matmuls into the
  sim PSUM (the first matmul of each 32-row group carries start=True).

HI chunk (p'=0..71 in free, partitions=i, free=(64j, p') per b-half):
  Q2PART (each partition = the flat q2-hi half-row) is built by a flatten
  DMA + 7 partition-doubling DMAs; the 8 chains are spread across the
  sync/gpsimd/scalar DMA queues and consumed LAST on the DVE stream so an
  unfinished chain never blocks in-order DVE progress. Subtract TT 2x
  (q1n-hi with j on a stride-0 middle dim), abs 4x, Reciprocal, then
  in-place fp16 tree folds 72->36->18->9 (TT 2x) + a small fp32
  tensor_reduce; [i,j-half] joins sim via a PE transpose-matmul.

The mul projection + att1 are spliced between att2 windows so the
in-order PE queue interleaves trunk matmuls with the ones-matmuls.
"""

import sys
from contextlib import ExitStack

import numpy as np

for _p in ("/opt/trn_rl_repo",):
    if _p not in sys.path:
        sys.path.insert(0, _p)

import concourse.bass as bass
import concourse.tile as tile
from concourse.bacc import Bacc
from concourse import mybir
from concourse.bass_utils import run_bass_kernel_spmd
from concourse.masks import make_identity

F32 = mybir.dt.float32
BF = mybir.dt.bfloat16
H16 = mybir.dt.float16
I16 = mybir.dt.int16
I32 = mybir.dt.int32
ALU = mybir.AluOpType
ACTF = mybir.ActivationFunctionType
AX = mybir.AxisListType

TRUNK = H16

B, S, D, P, V, C = 32, 128, 300, 200, 30000, 3
NCORES = 8
BL = B // NCORES  # 4 batch items per core
ROWS = BL * S  # 512 per side
ROWS2 = 2 * ROWS  # both sides in one trunk

CH_D = [(0, 128), (128, 128), (256, 44)]  # 300
CH_P = [(0, 128), (128, 72)]  # 200

PLO = 128  # att2 low p-chunk (partition dim)
PHI = 72  # att2 high p-chunk (free dim, layout B)
JB = 8  # j-block size for the LO path
NBLK = S // JB

WEIGHT_NAMES = [
    "hw1_Wh", "hw1_bh", "hw1_Wt", "hw1_bt",
    "hw2_Wh", "hw2_bh", "hw2_Wt", "hw2_bt",
    "mul_W1", "mul_b1", "mul_W2", "mul_b2",
    "dist_W1", "dist_b1", "dist_W2", "dist_b2",
    "cmp_W1", "cmp_b1", "cmp_W2", "cmp_b2",
    "chw1_Wh", "chw1_bh", "chw1_Wt", "chw1_bt",
    "chw2_Wh", "chw2_bh", "chw2_Wt", "chw2_bt",
    "agg_W1", "agg_b1", "agg_W2", "agg_b2",
    "out_W", "out_b",
]

F32_WEIGHTS = {"agg_W1", "agg_W2", "out_W"}


def _chunks(n):
    out = []
    o = 0
    while o < n:
        c = min(128, n - o)
        out.append((o, c))
        o += c
    return out


def act_recip(nc, out, in_, bias=0.0):
    """out = 1/(in_ + bias) in one ScalarE pass (Reciprocal activation)."""
    eng = nc.scalar
    ins_ = [
        eng.lower_ap(in_),
        mybir.ImmediateValue(dtype=mybir.dt.float32, value=bias),
        mybir.ImmediateValue(dtype=mybir.dt.float32, value=1.0),
        mybir.ImmediateValue(dtype=mybir.dt.float32, value=0.0),
    ]
    return eng.add_instruction(
        mybir.InstActivation(
            name=eng.bass.get_next_instruction_name(),
            func=ACTF.Reciprocal,
            ins=ins_,
            outs=[eng.lower_ap(out)],
        )
    )


def build_nc(debug=False):
    nc = Bacc()

    io = {}
    io["x1"] = nc.declare_dram_parameter("x1", [BL, S], I32, isOutput=False)
    io["x2"] = nc.declare_dram_parameter("x2", [BL, S], I32, isOutput=False)
    io["emb"] = nc.declare_dram_parameter("emb", [V, D], F32, isOutput=False)
    shapes = {
        "hw1_Wh": [D, D], "hw1_bh": [D], "hw1_Wt": [D, D], "hw1_bt": [D],
        "hw2_Wh": [D, D], "hw2_bh": [D], "hw2_Wt": [D, D], "hw2_bt": [D],
        "mul_W1": [D, P], "mul_b1": [P], "mul_W2": [P, P], "mul_b2": [P],
        "dist_W1": [D, P], "dist_b1": [P], "dist_W2": [P, P], "dist_b2": [P],
        "cmp_W1": [4 * D, P], "cmp_b1": [P], "cmp_W2": [P, P], "cmp_b2": [P],
        "chw1_Wh": [P, P], "chw1_bh": [P], "chw1_Wt": [P, P], "chw1_bt": [P],
        "chw2_Wh": [P, P], "chw2_bh": [P], "chw2_Wt": [P, P], "chw2_bt": [P],
        "agg_W1": [4 * P, P], "agg_b1": [P], "agg_W2": [P, P], "agg_b2": [P],
        "out_W": [P, C], "out_b": [C],
    }
    for n in WEIGHT_NAMES:
        io[n] = nc.declare_dram_parameter(n, shapes[n], F32, isOutput=False)
    io["yt"] = nc.declare_dram_parameter("yt", [C, BL], F32, isOutput=True)
    if debug:
        io["dbg_sim4"] = nc.declare_dram_parameter("dbg_sim4", [128, 512], F32, isOutput=True)

    with ExitStack() as ctx:
        tc = ctx.enter_context(tile.TileContext(nc))
        _emit(ctx, nc, tc, io, debug=debug)
    nc.finalize()
    return nc


def _emit(ctx, nc, tc, io, debug=False):
    wpool = ctx.enter_context(tc.tile_pool(name="wpool", bufs=1))
    wstage = ctx.enter_context(tc.tile_pool(name="wstage", bufs=2))
    const = ctx.enter_context(tc.tile_pool(name="const", bufs=1))
    persist = ctx.enter_context(tc.tile_pool(name="persist", bufs=1))
    work = ctx.enter_context(tc.tile_pool(name="work", bufs=1))
    small = ctx.enter_context(tc.tile_pool(name="small", bufs=3))

    pp_mm = ctx.enter_context(tc.tile_pool(name="pp_mm", bufs=3, space="PSUM"))
    pp_sim = ctx.enter_context(tc.tile_pool(name="pp_sim", bufs=1, space="PSUM"))
    pp_tr = ctx.enter_context(tc.tile_pool(name="pp_tr", bufs=2, space="PSUM"))
    pp_sm = ctx.enter_context(tc.tile_pool(name="pp_sm", bufs=2, space="PSUM"))

    # ---------------- constants ----------------
    identf = const.tile([128, 128], F32, tag="identf", name="identf")
    make_identity(nc, identf[:, :])
    identr = const.tile([128, 128], TRUNK, tag="identr", name="identr")
    nc.vector.tensor_scalar_add(out=identr[:, :], in0=identf[:, :], scalar1=0.0)
    identb = const.tile([128, 128], H16, tag="identb", name="identb")
    nc.vector.tensor_scalar_add(out=identb[:, :], in0=identf[:, :], scalar1=0.0)

    # sliding ones-column buffer: Z[:, 32] == 1 so Z[:, 32-r:64-r] has its
    # ones in column r; Z_slice.T @ U deposits column-sums of U into row r.
    zbuf = const.tile([128, 64], H16, tag="zbuf", name="zbuf")
    nc.vector.memset(zbuf[:, :], 0.0)
    nc.vector.memset(zbuf[:, 32:33], 1.0)

    # ---------------- weights: casting DMAs ----------------
    SPECIAL_KCH = {
        "cmp_W1": [(s * D + o, c) for s in range(4) for (o, c) in CH_D],
        "agg_W1": [(s * P + o, c) for s in range(4) for (o, c) in CH_P],
    }

    def load_w(name):
        h = io[name]
        K, M = h.shape
        dt = F32 if name in F32_WEIGHTS else H16
        tiles = []
        for i, (o, c) in enumerate(SPECIAL_KCH.get(name, _chunks(K))):
            t = wpool.tile([c, M], dt, tag=f"w_{name}_{i}", name=f"w_{name}_{i}")
            if dt == F32:
                nc.sync.dma_start(out=t[:, :], in_=h[o:o + c, :])
            else:
                stg = wstage.tile([128, M], F32, tag="wstg", name=f"wstg_{name}_{i}")
                nc.sync.dma_start(out=stg[:c, :], in_=h[o:o + c, :])
                nc.vector.tensor_scalar_add(out=t[:, :], in0=stg[:c, :], scalar1=0.0)
            tiles.append(t)
        return tiles

    def load_b(name):
        h = io[name]
        (M,) = h.shape
        tiles = []
        for i, (o, c) in enumerate(_chunks(M)):
            t = wpool.tile([c, 1], F32, tag=f"b_{name}_{i}", name=f"b_{name}_{i}")
            nc.sync.dma_start(out=t[:, :], in_=h[o:o + c])
            tiles.append(t)
        return tiles

    # ---------------- index DMAs + gathers (overlap weight DMAs) ----------
    pre2 = ctx.enter_context(ExitStack())
    g2pool = pre2.enter_context(tc.tile_pool(name="g2pool", bufs=1))
    pre1 = ctx.enter_context(ExitStack())
    gpool = pre1.enter_context(tc.tile_pool(name="gpool", bufs=1))
    e_n = {}
    for side, xh in (("1", io["x1"]), ("2", io["x2"])):
        for b in range(BL):
            idx = gpool.tile([128, 1], I32, tag=f"idx{side}_{b}", name=f"idx{side}_{b}")
            nc.sync.dma_start(out=idx[:, :], in_=xh[b, :])
            e = gpool.tile([128, D], H16, tag=f"e{side}_{b}", name=f"e{side}_{b}")
            nc.gpsimd.indirect_dma_start(
                out=e[:, :], out_offset=None, in_=io["emb"][:, :],
                in_offset=bass.IndirectOffsetOnAxis(ap=idx[:, :1], axis=0),
            )
            e_n[(side, b)] = e

    W = {}
    for n in WEIGHT_NAMES:
        W[n] = load_b(n) if n.endswith(("bh", "bt", "b1", "b2", "_b")) else load_w(n)

    # ---------------- helpers ----------------
    def mm_apply(w_tiles, b_tiles, rhs_tiles, n_free, func, out_tiles,
                 krange=None, mrange=None):
        """out = func(W.T @ rhs + b), transposed layout, 512-col PSUM chunks."""
        M = w_tiles[0].shape[1]
        mch = _chunks(M)
        ks = list(range(len(w_tiles))) if krange is None else krange
        m_iter = ([(i, i) for i in range(len(mch))] if mrange is None
                  else list(enumerate(mrange)))
        for oi, mi in m_iter:
            mo, mc = mch[mi]
            for fo in range(0, n_free, 512):
                fc = min(512, n_free - fo)
                ps = pp_mm.tile([128, 512], F32, tag="mmout", name="mmout")
                for idx, ki in enumerate(ks):
                    kc = w_tiles[ki].shape[0]
                    nc.tensor.matmul(
                        out=ps[:mc, :fc],
                        lhsT=w_tiles[ki][:kc, mo:mo + mc],
                        rhs=rhs_tiles[ki][:kc, fo:fo + fc],
                        start=(idx == 0),
                        stop=(idx == len(ks) - 1),
                    )
                nc.scalar.activation(
                    out=out_tiles[oi][:mc, fo:fo + fc],
                    in_=ps[:mc, :fc],
                    func=func, bias=b_tiles[mi][:mc, :], scale=1.0,
                )

    def highway(xt_tiles, wh, bh, wt, bt, feat, out_tiles):
        """out = x + t*(h-x), trunk layout, chunk-at-a-time (h reused as tmp)."""
        ch = _chunks(feat)
        for mi, (mo, mc) in enumerate(ch):
            h = work.tile([128, ROWS2], TRUNK, tag=f"hw_h{mi % 2}", name="hw_h")
            t = work.tile([128, ROWS2], TRUNK, tag=f"hw_t{mi % 2}", name="hw_t")
            mm_apply(wh, bh, xt_tiles, ROWS2, ACTF.Relu, [h], mrange=[mi])
            mm_apply(wt, bt, xt_tiles, ROWS2, ACTF.Sigmoid, [t], mrange=[mi])
            x_sl = xt_tiles[mi][:mc, :]
            nc.vector.tensor_tensor(out=h[:mc, :], in0=h[:mc, :], in1=x_sl,
                                    op=ALU.subtract)
            nc.vector.tensor_tensor(out=h[:mc, :], in0=h[:mc, :], in1=t[:mc, :],
                                    op=ALU.mult)
            nc.vector.tensor_tensor(out=out_tiles[mi][:mc, :], in0=h[:mc, :],
                                    in1=x_sl, op=ALU.add)

    # ---------------- embed: transpose into trunk ----------------
    eT = [g2pool.tile([128, ROWS2], TRUNK, tag=f"eT_{i}", name=f"eT_{i}")
          for i in range(3)]
    for ki, (ko, kc) in enumerate(CH_D):
        for side in ("1", "2"):
            ps = pp_tr.tile([128, 512], H16, tag="trpackb", name="trpack")
            for b in range(BL):
                nc.tensor.transpose(
                    out=ps[:kc, b * S:(b + 1) * S],
                    in_=e_n[(side, b)][:, ko:ko + kc],
                    identity=identb[:128, :128],
                )
            so = (0 if side == "1" else ROWS)
            nc.scalar.activation(out=eT[ki][:kc, so:so + ROWS], in_=ps[:kc, :ROWS],
                                 func=ACTF.Copy)
    pre1.close()  # frees index + gather tiles

    # ---------------- highway stack (trunk: both sides at once) -------------
    h1 = [g2pool.tile([128, ROWS2], TRUNK, tag=f"hwy1_{i}", name=f"hwy1_{i}")
          for i in range(3)]
    highway(eT, W["hw1_Wh"], W["hw1_bh"], W["hw1_Wt"], W["hw1_bt"], D, h1)
    eTh = [persist.tile([128, ROWS2], TRUNK, tag=f"eTh_{i}", name=f"eTh_{i}")
           for i in range(3)]
    highway(h1, W["hw2_Wh"], W["hw2_bh"], W["hw2_Wt"], W["hw2_bt"], D, eTh)
    pre2.close()  # frees eT, h1

    # ---------------- projections (shared weights, trunk) ----------------
    def proj(prefix, pool):
        z1 = [work.tile([128, ROWS2], TRUNK, tag=f"z1_{i}", name=f"z1_{i}") for i in range(2)]
        mm_apply(W[f"{prefix}_W1"], W[f"{prefix}_b1"], eTh, ROWS2, ACTF.Relu, z1)
        out = [pool.tile([128, ROWS2], TRUNK, tag=f"{prefix}T_{i}", name=f"{prefix}T_{i}")
               for i in range(2)]
        mm_apply(W[f"{prefix}_W2"], W[f"{prefix}_b2"], z1, ROWS2, ACTF.Relu, out)
        return out

    # dist first so the att2 elementwise can start while the PE continues
    # with the mul projection; hi chunk (m=1) first for earlier hi-prep
    qT = proj("dist", persist)
    # qT[0]: p 0..127 [128, (side,b,t)]; qT[1][:72]: p 128..199

    # ---- att2 prep: hi-chunk transposes to normal layout, per b ----
    # (pools opened here, after the gather/highway scratch is freed)
    q2ppool = ctx.enter_context(tc.tile_pool(name="q2ppool", bufs=8))
    uhipool = ctx.enter_context(tc.tile_pool(name="uhipool", bufs=2))
    q2rpool = ctx.enter_context(tc.tile_pool(name="q2rpool", bufs=2))
    ulopool = ctx.enter_context(tc.tile_pool(name="ulopool", bufs=3))
    shpool = ctx.enter_context(tc.tile_pool(name="shpool", bufs=2))
    q1n_hi, q2part, q2nh = {}, {}, {}
    for b in range(BL):
        ps = pp_tr.tile([128, 512], H16, tag="trpackb", name=f"hitr_{b}")
        nc.tensor.transpose(out=ps[:128, 0:PHI],
                            in_=qT[1][:PHI, b * S:(b + 1) * S],
                            identity=identr[:PHI, :PHI])
        nc.tensor.transpose(out=ps[:128, 128:128 + PHI],
                            in_=qT[1][:PHI, ROWS + b * S:ROWS + (b + 1) * S],
                            identity=identr[:PHI, :PHI])
        t1 = persist.tile([128, PHI], H16, tag=f"q1nh_{b}", name=f"q1nh_{b}")
        nc.vector.tensor_scalar_add(out=t1[:, :], in0=ps[:128, 0:PHI], scalar1=0.0)
        q1n_hi[b] = t1
        t2 = persist.tile([128, PHI], H16, tag=f"q2nh_{b}", name=f"q2nh_{b}")
        nc.vector.tensor_scalar_add(out=t2[:, :], in0=ps[:128, 128:128 + PHI],
                                    scalar1=0.0)
        q2nh[b] = t2

    # Q2PART[b,h]: [128 i, (64 j, p')], every partition = the flat q2n_hi
    # half-row; flatten seed + 7 doubling rounds. Emitted ROUND-MAJOR
    # across all 8 chains: each DMA's dependency was issued 8 slots
    # earlier, so the in-order sync queue never blocks on a waiting head.
    SHH = 64
    for b in range(BL):
        for h in range(2):
            q2part[(b, h)] = q2ppool.tile([128, SHH * PHI], H16, tag="q2part",
                                          name=f"q2p_{b}_{h}")
    for b in range(BL):
        for h in range(2):
            qp = q2part[(b, h)]
            qeng = (nc.sync, nc.sync, nc.gpsimd, nc.scalar)[b]
            qeng.dma_start(out=qp[0:1, :],
                           in_=q2nh[b][h * SHH:(h + 1) * SHH, :])
            n = 1
            while n < 128:
                qeng.dma_start(out=qp[n:2 * n, :], in_=qp[0:n, :])
                n *= 2

    # normal-layout post-highway embeddings (lhsT for the beta/alpha
    # matmuls); PE runs these during its att2 idle windows.
    ehw_n = {}
    for side in ("1", "2"):
        so = (0 if side == "1" else ROWS)
        for b in range(BL):
            ps = pp_tr.tile([128, 512], H16, tag="trpackb", name="trpackr")
            for ki, (ko, kc) in enumerate(CH_D):
                nc.tensor.transpose(
                    out=ps[:128, ko:ko + kc],
                    in_=eTh[ki][:kc, so + b * S:so + (b + 1) * S],
                    identity=identr[:kc, :kc],
                )
            t = persist.tile([128, D], H16, tag=f"ehwn{side}_{b}", name=f"ehwn{side}_{b}")
            nc.scalar.activation(out=t[:, :], in_=ps[:, :D], func=ACTF.Copy)
            ehw_n[(side, b)] = t

    # sim4 PSUM accumulates (in PE order): lo ones-matmuls + hi transposes
    # (emitted in the att2 loop, first one carries start=True) + att1
    # (emitted between windows once the mul projection is done).
    sim4 = pp_sim.tile([128, 512], F32, tag="sim4", name="sim4")
    sim4_group_started = [False] * 4

    # ---------------- att2 ----------------
    q1lo = qT[0][:PLO, 0:ROWS]  # [p, (b,i)]
    q2lo = qT[0][:PLO, ROWS:ROWS2]  # [p, (b,j)]

    W8 = 8  # replicated q2 stripe width for the LO subtract

    def raw_tt(out, in0, in1, op):
        """TensorTensor with shape-mismatched APs (same element order)."""
        eng = nc.vector
        return eng.add_instruction(
            mybir.InstTensorTensor(
                name=eng.bass.get_next_instruction_name(),
                op=op,
                ins=[eng.lower_ap(in0), eng.lower_ap(in1)],
                outs=[eng.lower_ap(out)],
            )
        )

    def lo_block(jb, abs_on_scalar=False):
        """LO p-chunk, j-block jb: u[p, (b, 8j, 128i)] -> ones-matmuls."""
        ncols = BL * JB * S  # 4096
        # q2r8: q2lo replicated along an i-stripe of width 8 (one 1x DVE
        # seed, 256 cols), laid out (b, j, i8). The subtract reads it with
        # a stride-0 MIDDLE dim (i-group): all inner dims contiguous -> 2x.
        q2r = q2rpool.tile([128, BL * JB * W8], H16, tag="q2r", name=f"q2r_{jb}")
        seed_in = bass.AP(
            tensor=q2lo.tensor, offset=q2lo.offset + jb * JB,
            ap=[q2lo.ap[0], [S, BL], [1, JB], [0, W8]])
        seed_out = bass.AP(
            tensor=q2r.tensor, offset=q2r.offset,
            ap=[q2r.ap[0], [JB * W8, BL], [W8, JB], [1, W8]])
        nc.vector.tensor_scalar_add(out=seed_out, in0=seed_in, scalar1=0.0)
        # subtract (TT 2x), ONE inst per block: both APs enumerate
        # (b, j, i) in the same order but with different dim structures
        # (bass's free-shape assert is bypassed; the ISA only streams).
        u = ulopool.tile([128, ncols], H16, tag="ulo", name=f"ulo_{jb}")
        in0 = bass.AP(tensor=q1lo.tensor, offset=q1lo.offset,
                      ap=[q1lo.ap[0], [S, BL], [0, JB], [1, S]])
        in1 = bass.AP(tensor=q2r.tensor, offset=q2r.offset,
                      ap=[q2r.ap[0], [W8, BL * JB], [0, S // W8], [1, W8]])
        uo = bass.AP(tensor=u.tensor, offset=u.offset,
                     ap=[u.ap[0], [1, ncols]])
        raw_tt(uo, in0, in1, ALU.subtract)
        if abs_on_scalar:
            # |u| then 1/(1+|u|), both on ScalarE (DVE relief)
            nc.scalar.activation(out=u[:, :], in_=u[:, :], func=ACTF.Abs)
        else:
            ui = u[:128, :].bitcast(I16)
            nc.vector.tensor_scalar(out=ui, in0=ui, scalar1=0x7FFF, scalar2=None,
                                    op0=ALU.bitwise_and)
        act_recip(nc, u[:, :], u[:, :], bias=1.0)
        # partition sums into sim4 rows via sliding ones-columns
        for jj in range(JB):
            j = jb * JB + jj
            g, rr = j // 32, j % 32
            rbase = u[:128, jj * S:jj * S + S]
            rhs = bass.AP(tensor=rbase.tensor, offset=rbase.offset,
                          ap=[rbase.ap[0], [JB * S, BL], [1, S]])
            nc.tensor.matmul(
                out=sim4[32 * g:32 * g + 32, :],
                lhsT=zbuf[:128, 32 - rr:64 - rr],
                rhs=rhs,
                start=(not sim4_group_started[g]), stop=False,
                skip_group_check=True,
                tile_position=(0, 32 * g),
            )
            sim4_group_started[g] = True

    SH = S // 2  # hi-path j-half size
    hi_done = []

    def hi_half(b, h):
        """HI p-chunk, batch b, j-half h: u[i, (64j, p')] -> tree -> transpose."""
        ncols = SH * PHI  # 4608
        qp = q2part[(b, h)]
        u = uhipool.tile([128, ncols], H16, tag="uhi", name=f"uhi_{b}_{h}")
        in0 = bass.AP(tensor=q1n_hi[b].tensor, offset=q1n_hi[b].offset,
                      ap=[q1n_hi[b].ap[0], [0, SH], [1, PHI]])
        u3 = u[:128, :].rearrange("i (j p) -> i j p", j=SH)
        qp3 = qp[:128, :].rearrange("i (j p) -> i j p", j=SH)
        nc.vector.tensor_tensor(out=u3, in0=in0, in1=qp3, op=ALU.subtract)
        ui = u[:128, :].bitcast(I16)
        nc.vector.tensor_scalar(out=ui, in0=ui, scalar1=0x7FFF, scalar2=None,
                                op0=ALU.bitwise_and)
        act_recip(nc, u[:, :], u[:, :], bias=1.0)
        # in-place fp16 tree folds: 72 -> 36 -> 18 (TT 2x)
        nc.vector.tensor_tensor(out=u3[:, :, 0:36], in0=u3[:, :, 0:36],
                                in1=u3[:, :, 36:72], op=ALU.add)
        nc.vector.tensor_tensor(out=u3[:, :, 0:18], in0=u3[:, :, 0:18],
                                in1=u3[:, :, 18:36], op=ALU.add)
        nc.vector.tensor_tensor(out=u3[:, :, 0:9], in0=u3[:, :, 0:9],
                                in1=u3[:, :, 9:18], op=ALU.add)
        sh = shpool.tile([128, SH], F32, tag="simhi", name=f"simhi_{b}_{h}")
        nc.vector.tensor_reduce(out=sh[:, :], in_=u3[:, :, 0:9], axis=AX.X,
                                op=ALU.add)
        shh = persist.tile([128, SH], H16, tag=f"simhih_{b}_{h}",
                           name=f"simhih_{b}_{h}")
        nc.vector.tensor_scalar_add(out=shh[:, :], in0=sh[:, :], scalar1=0.0)
        hi_done.append((b, h, shh))

    # interleave: att2 windows with the mul-projection pieces + att1 (PE
    # executes its queue in order, so trunk matmuls are spliced between
    # the windows' ones-matmuls).
    z1m = [work.tile([128, ROWS2], TRUNK, tag=f"z1_{i}", name=f"z1m_{i}")
           for i in range(2)]
    pT = [persist.tile([128, ROWS2], TRUNK, tag=f"mulT_{i}", name=f"mulT_{i}")
          for i in range(2)]

    def mul_piece(k):
        if k < 2:
            mm_apply(W["mul_W1"], W["mul_b1"], eTh, ROWS2, ACTF.Relu,
                     [z1m[k]], mrange=[k])
        else:
            mm_apply(W["mul_W2"], W["mul_b2"], z1m, ROWS2, ACTF.Relu,
                     [pT[k - 2]], mrange=[k - 2])

    ABS_SCALAR_BLOCKS = set()
    # all lo blocks first: they have no DMA dependency, so the in-order
    # DVE stream never stalls on a Q2PART chain while lo work remains
    for b in range(BL):
        for jb in range(4 * b, 4 * b + 4):
            lo_block(jb, abs_on_scalar=(jb in ABS_SCALAR_BLOCKS))
        mul_piece(b)
    # hi halves last, in chain-completion order
    for b in range(BL):
        for h in range(2):
            hi_half(b, h)
    # att1
    for bb in range(BL):
        for ki, (ko, kc) in enumerate(CH_P):
            nc.tensor.matmul(
                out=sim4[:, bb * S:(bb + 1) * S],
                lhsT=pT[ki][:kc, ROWS + bb * S:ROWS + (bb + 1) * S],
                rhs=pT[ki][:kc, bb * S:(bb + 1) * S],
                start=False, stop=False, skip_group_check=True,
            )
    # deferred hi transpose-accumulates [i, j-half] -> sim4
    for (bb, h, shh) in hi_done:
        nc.tensor.matmul(
            out=sim4[h * SH:(h + 1) * SH, bb * S:(bb + 1) * S],
            lhsT=shh[:, :], rhs=identb[:, :],
            start=False, stop=False, skip_group_check=True,
            tile_position=(0, h * SH),
        )

    # close the sim4 accumulation group with a dummy stopping matmul on a
    # zero rhs? Not needed: readers below read PSUM after all writers.

    if debug and "dbg_sim4" in io:
        t = small.tile([128, 512], F32, tag="dbgps", name="dbg_sim4_t")
        nc.scalar.activation(out=t[:, :], in_=sim4[:, :], func=ACTF.Copy)
        nc.gpsimd.dma_start(out=io["dbg_sim4"][:, :], in_=t[:, :])

    # ---------------- softmax + beta/alpha + compare part 1 ----------------
    def softmax_p(src_psum):
        """softmax over rows of src [128,128]; returns transposed probs fp16."""
        mx = small.tile([128, 1], F32, tag="sm_mx", name="sm_mx")
        nc.vector.tensor_reduce(out=mx[:, :], in_=src_psum, axis=AX.X,
                                op=ALU.max, negate=True)
        esb = small.tile([128, S], H16, tag="sm_e", name="sm_e")
        zs = small.tile([128, 1], F32, tag="sm_z", name="sm_z")
        nc.scalar.activation(out=esb[:, :], in_=src_psum, func=ACTF.Exp,
                             bias=mx[:, :], scale=1.0, accum_out=zs[:, :])
        rz = small.tile([128, 1], F32, tag="sm_rz", name="sm_rz")
        nc.vector.reciprocal(out=rz[:, :], in_=zs[:, :])
        pr = small.tile([128, S], H16, tag="sm_p", name="sm_p")
        nc.vector.tensor_scalar(out=pr[:, :], in0=esb[:, :], scalar1=rz[:, :],
                                scalar2=None, op0=ALU.mult)
        pt_ps = pp_tr.tile([128, 512], H16, tag="trpackb", name="trpackb")
        nc.tensor.transpose(out=pt_ps[:S, :S], in_=pr[:, :], identity=identb[:, :])
        pt = small.tile([128, S], H16, tag="sm_pt", name="sm_pt")
        nc.vector.tensor_scalar_add(out=pt[:, :], in0=pt_ps[:S, :S], scalar1=0.0)
        return pt

    betaT = {s: [persist.tile([128, 512], H16, tag=f"betaT{s}_{i}", name=f"betaT{s}_{i}")
                 for i in range(3)] for s in ("1", "2")}

    for b in range(BL):
        bs4 = sim4[:, b * S:(b + 1) * S]
        simT_sb = small.tile([128, S], F32, tag="simT_sb", name="simT_sb")
        nc.vector.tensor_scalar_add(out=simT_sb[:, :], in0=bs4, scalar1=0.0)
        sim_ps = pp_sm.tile([128, S], F32, tag="btps", name="simtr")
        nc.tensor.transpose(out=sim_ps[:S, :S], in_=simT_sb[:, :],
                            identity=identf[:, :])
        ptA = softmax_p(bs4)  # alpha probs^T [i, j]
        ptB = softmax_p(sim_ps[:S, :S])  # beta probs^T [j, i]

        for side, pt, eln in (("2", ptA, "1"), ("1", ptB, "2")):
            for ki, (ko, kc) in enumerate(CH_D):
                bt_ps = pp_sm.tile([128, S], F32, tag="btps", name="btps")
                nc.tensor.matmul(
                    out=bt_ps[:kc, :], lhsT=ehw_n[(eln, b)][:, ko:ko + kc],
                    rhs=pt[:, :], start=True, stop=True,
                )
                nc.vector.tensor_scalar_add(
                    out=betaT[side][ki][:kc, b * S:(b + 1) * S],
                    in0=bt_ps[:kc, :], scalar1=0.0)

    # cat + compare matmul, per side over 512-col trunk halves.
    cmp1 = [persist.tile([128, ROWS2], H16, tag=f"cmp1_{i}", name=f"cmp1_{i}")
            for i in range(2)]
    for side in ("1", "2"):
        so = (0 if side == "1" else ROWS)
        ps2 = [pp_mm.tile([128, 512], F32, tag="mmout", name=f"cmp1ps{mi}")
               for mi in range(2)]
        for sel in range(4):  # e, beta, e-beta, e*beta
            for ki, (ko, kc) in enumerate(CH_D):
                e_sl = eTh[ki][:kc, so:so + ROWS]
                b_sl = betaT[side][ki][:kc, :]
                if sel == 0:
                    rhs = e_sl  # pure-e chunk: use eTh directly, no copy
                elif sel == 1:
                    rhs = b_sl
                else:
                    cat = small.tile([128, 512], H16, tag="cat", name="cat")
                    nc.vector.tensor_tensor(
                        out=cat[:kc, :], in0=e_sl, in1=b_sl,
                        op=(ALU.subtract if sel == 2 else ALU.mult))
                    rhs = cat[:kc, :]
                idx = sel * 3 + ki
                for mi, (mo, mc) in enumerate(CH_P):
                    nc.tensor.matmul(
                        out=ps2[mi][:mc, :],
                        lhsT=W["cmp_W1"][idx][:, mo:mo + mc],
                        rhs=rhs,
                        start=(idx == 0), stop=(idx == 11),
                        skip_group_check=True,
                    )
        for mi, (mo, mc) in enumerate(CH_P):
            nc.scalar.activation(
                out=cmp1[mi][:mc, so:so + ROWS], in_=ps2[mi][:mc, :],
                func=ACTF.Relu,
                bias=W["cmp_b1"][mi][:mc, :], scale=1.0,
            )

    # ---------------- compare part 2 + compare highway (trunk) --------------
    v0 = [work.tile([128, ROWS2], H16, tag=f"z1_{i}", name=f"v0_{i}") for i in range(2)]
    mm_apply(W["cmp_W2"], W["cmp_b2"], cmp1, ROWS2, ACTF.Relu, v0)
    v1 = [work.tile([128, ROWS2], H16, tag=f"v1_{i}", name=f"v1_{i}") for i in range(2)]
    highway(v0, W["chw1_Wh"], W["chw1_bh"], W["chw1_Wt"], W["chw1_bt"], P, v1)
    vT = [persist.tile([128, ROWS2], H16, tag=f"vT_{i}", name=f"vT_{i}")
          for i in range(2)]
    highway(v1, W["chw2_Wh"], W["chw2_bh"], W["chw2_Wt"], W["chw2_bt"], P, vT)

    # ---------------- aggregate (fp32) ----------------
    stats = []
    for sect, (side, op) in enumerate(
            (("1", ALU.max), ("2", ALU.max), ("1", ALU.add), ("2", ALU.add))):
        so = (0 if side == "1" else ROWS)
        st = [persist.tile([128, BL], F32, tag=f"st{sect}_{i}", name=f"st{sect}_{i}")
              for i in range(2)]
        for ki, (ko, kc) in enumerate(CH_P):
            seg = vT[ki][:kc, so:so + ROWS].rearrange("p (b t) -> p b t", b=BL)
            nc.vector.tensor_reduce(
                out=st[ki][:kc, :BL], in_=seg, axis=AX.X, op=op,
            )
        stats.append(st)

    agg_rhs = [stats[s][ki] for s in range(4) for ki in range(2)]
    y1 = [persist.tile([128, BL], F32, tag=f"y1_{i}", name=f"y1_{i}") for i in range(2)]
    mm_apply(W["agg_W1"], W["agg_b1"], agg_rhs, BL, ACTF.Relu, y1)
    y2 = [persist.tile([128, BL], F32, tag=f"y2_{i}", name=f"y2_{i}") for i in range(2)]
    mm_apply(W["agg_W2"], W["agg_b2"], y1, BL, ACTF.Relu, y2)

    yt_ps = pp_sm.tile([128, S], F32, tag="btps", name="btps")
    for ki, (ko, kc) in enumerate(CH_P):
        nc.tensor.matmul(
            out=yt_ps[:C, :BL], lhsT=W["out_W"][ki][:kc, :],
            rhs=y2[ki][:kc, :], start=(ki == 0), stop=(ki == 1),
        )
    yt_sb = persist.tile([C, BL], F32, tag="yt_sb", name="yt_sb")
    nc.scalar.activation(out=yt_sb[:, :], in_=yt_ps[:C, :BL], func=ACTF.Identity,
                         bias=W["out_b"][0][:C, :], scale=1.0)
    nc.sync.dma_start(out=io["yt"][:, :], in_=yt_sb[:, :])


_NC_CACHE = {}


def _get_nc():
    if "nc" not in _NC_CACHE:
        _NC_CACHE["nc"] = build_nc()
    return _NC_CACHE["nc"]


def make_in_maps(inputs):
    """Shard full inputs into 8 per-core input maps."""
    x1 = np.ascontiguousarray(np.asarray(inputs["x1"]).astype(np.int32))
    x2 = np.ascontiguousarray(np.asarray(inputs["x2"]).astype(np.int32))
    shared = {}
    for n in WEIGHT_NAMES + ["emb"]:
        shared[n] = np.ascontiguousarray(np.asarray(inputs[n]).astype(np.float32))
    in_maps = []
    for c in range(NCORES):
        m = dict(shared)
        m["x1"] = x1[c * BL:(c + 1) * BL]
        m["x2"] = x2[c * BL:(c + 1) * BL]
        in_maps.append(m)
    return in_maps


def kernel(**inputs):
    nc = _get_nc()
    in_maps = make_in_maps(inputs)
    res = run_bass_kernel_spmd(nc, in_maps, core_ids=list(range(NCORES)))
    return np.concatenate([np.asarray(r["yt"]).T for r in res.results], axis=0)


if __name__ == "__main__":
    nc = build_nc()
    print("built ok")


# revision 36
# speedup vs baseline: 1.1123x; 1.0197x over previous
"""Trainium2 Bass kernel for nn_AttentiveModel (B=32,S=128,D=300,P=200,V=30000,C=3).

Data-parallel over batch across 8 NeuronCores (4 batch items per core, all
weights replicated). Trunk compute (highways/projections/compare) runs in
fp16 on the PE; the dist-attention (att2) elementwise is structured so every
bulk DVE op hits a 2x/4x perf mode: those modes require all operands'
INNERMOST access-pattern dim to be stride +-1 / 2-byte, so broadcasts are
placed on middle dims or pre-replicated.

att2[b,j,i] = sum_p 1/(1+|q1[b,i,p]-q2[b,j,p]|), split by p-chunk:

LO chunk (p=0..127 on partitions, free=(b, 8j, 128i) per j-block):
  q2 is replicated only along an 8-wide i-stripe (one DVE seed inst); the
  subtract is a single raw TensorTensor whose operands use different dim
  structures enumerating the same (b,j,i) element order (q1 with j on a
  stride-0 middle dim; q2r8 with a stride-0 i-group dim) -> 2x. Abs is a
  4x tensor_scalar bitwise_and on the int16 view; ScalarE Reciprocal
  (bias=1); partition sums via sliding ones-column PE matmuls into the
  sim PSUM (the first matmul of each 32-row group carries start=True).

HI chunk (p'=0..71 in free, partitions=i, free=(64j, p') per b-half):
  Q2PART (each partition = the flat q2-hi half-row) is built by a flatten
  DMA + 7 partition-doubling DMAs; the 8 chains are spread across the
  sync/gpsimd/scalar DMA queues and consumed LAST on the DVE stream so an
  unfinished chain never blocks in-order DVE progress. Subtract TT 2x
  (q1n-hi with j on a stride-0 middle dim), abs 4x, Reciprocal, then
  in-place fp16 tree folds 72->36->18->9 (TT 2x) + a small fp32
  tensor_reduce; [i,j-half] joins sim via a PE transpose-matmul.

The mul projection + att1 are spliced between att2 windows so the
in-order PE queue interleaves trunk matmuls with the ones-matmuls.
"""

import sys
from contextlib import ExitStack

import numpy as np

for _p in ("/opt/trn_rl_repo",):
    if _p not in sys.path:
        sys.path.insert(0, _p)

import concourse.bass as bass
import concourse.tile as tile
from concourse.bacc import Bacc
from concourse import mybir
from concourse.bass_utils import run_bass_kernel_spmd
from concourse.masks import make_identity

F32 = mybir.dt.float32
BF = mybir.dt.bfloat16
H16 = mybir.dt.float16
I16 = mybir.dt.int16
I32 = mybir.dt.int32
ALU = mybir.AluOpType
ACTF = mybir.ActivationFunctionType
AX = mybir.AxisListType

TRUNK = H16

B, S, D, P, V, C = 32, 128, 300, 200, 30000, 3
NCORES = 8
BL = B // NCORES  # 4 batch items per core
ROWS = BL * S  # 512 per side
ROWS2 = 2 * ROWS  # both sides in one trunk

CH_D = [(0, 128), (128, 128), (256, 44)]  # 300
CH_P = [(0, 128), (128, 72)]  # 200

PLO = 128  # att2 low p-chunk (partition dim)
PHI = 72  # att2 high p-chunk (free dim, layout B)
JB = 8  # j-block size for the LO path
NBLK = S // JB

WEIGHT_NAMES = [
    "hw1_Wh", "hw1_bh", "hw1_Wt", "hw1_bt",
    "hw2_Wh", "hw2_bh", "hw2_Wt", "hw2_bt",
    "mul_W1", "mul_b1", "mul_W2", "mul_b2",
    "dist_W1", "dist_b1", "dist_W2", "dist_b2",
    "cmp_W1", "cmp_b1", "cmp_W2", "cmp_b2",
    "chw1_Wh", "chw1_bh", "chw1_Wt", "chw1_bt",
    "chw2_Wh", "chw2_bh", "chw2_Wt", "chw2_bt",
    "agg_W1", "agg_b1", "agg_W2", "agg_b2",
    "out_W", "out_b",
]

F32_WEIGHTS = {"agg_W1", "agg_W2", "out_W"}


def _chunks(n):
    out = []
    o = 0
    while o < n:
        c = min(128, n - o)
        out.append((o, c))
        o += c
    return out


def act_recip(nc, out, in_, bias=0.0):
    """out = 1/(in_ + bias) in one ScalarE pass (Reciprocal activation)."""
    eng = nc.scalar
    ins_ = [
        eng.lower_ap(in_),
        mybir.ImmediateValue(dtype=mybir.dt.float32, value=bias),
        mybir.ImmediateValue(dtype=mybir.dt.float32, value=1.0),
        mybir.ImmediateValue(dtype=mybir.dt.float32, value=0.0),
    ]
    return eng.add_instruction(
        mybir.InstActivation(
            name=eng.bass.get_next_instruction_name(),
            func=ACTF.Reciprocal,
            ins=ins_,
            outs=[eng.lower_ap(out)],
        )
    )


def build_nc(debug=False):
    nc = Bacc()

    io = {}
    io["x1"] = nc.declare_dram_parameter("x1", [BL, S], I32, isOutput=False)
    io["x2"] = nc.declare_dram_parameter("x2", [BL, S], I32, isOutput=False)
    io["emb"] = nc.declare_dram_parameter("emb", [V, D], F32, isOutput=False)
    shapes = {
        "hw1_Wh": [D, D], "hw1_bh": [D], "hw1_Wt": [D, D], "hw1_bt": [D],
        "hw2_Wh": [D, D], "hw2_bh": [D], "hw2_Wt": [D, D], "hw2_bt": [D],
        "mul_W1": [D, P], "mul_b1": [P], "mul_W2": [P, P], "mul_b2": [P],
        "dist_W1": [D, P], "dist_b1": [P], "dist_W2": [P, P], "dist_b2": [P],
        "cmp_W1": [4 * D, P], "cmp_b1": [P], "cmp_W2": [P, P], "cmp_b2": [P],
        "chw1_Wh": [P, P], "chw1_bh": [P], "chw1_Wt": [P, P], "chw1_bt": [P],
        "chw2_Wh": [P, P], "chw2_bh": [P], "chw2_Wt": [P, P], "chw2_bt": [P],
        "agg_W1": [4 * P, P], "agg_b1": [P], "agg_W2": [P, P], "agg_b2": [P],
        "out_W": [P, C], "out_b": [C],
    }
    for n in WEIGHT_NAMES:
        io[n] = nc.declare_dram_parameter(n, shapes[n], F32, isOutput=False)
    io["yt"] = nc.declare_dram_parameter("yt", [C, BL], F32, isOutput=True)
    if debug:
        io["dbg_sim4"] = nc.declare_dram_parameter("dbg_sim4", [128, 512], F32, isOutput=True)

    with ExitStack() as ctx:
        tc = ctx.enter_context(tile.TileContext(nc))
        _emit(ctx, nc, tc, io, debug=debug)
    nc.finalize()
    return nc


def _emit(ctx, nc, tc, io, debug=False):
    wpool = ctx.enter_context(tc.tile_pool(name="wpool", bufs=1))
    wstage = ctx.enter_context(tc.tile_pool(name="wstage", bufs=2))
    const = ctx.enter_context(tc.tile_pool(name="const", bufs=1))
    persist = ctx.enter_context(tc.tile_pool(name="persist", bufs=1))
    work = ctx.enter_context(tc.tile_pool(name="work", bufs=1))
    small = ctx.enter_context(tc.tile_pool(name="small", bufs=3))

    pp_mm = ctx.enter_context(tc.tile_pool(name="pp_mm", bufs=3, space="PSUM"))
    pp_sim = ctx.enter_context(tc.tile_pool(name="pp_sim", bufs=1, space="PSUM"))
    pp_tr = ctx.enter_context(tc.tile_pool(name="pp_tr", bufs=2, space="PSUM"))
    pp_sm = ctx.enter_context(tc.tile_pool(name="pp_sm", bufs=2, space="PSUM"))

    # ---------------- constants ----------------
    identf = const.tile([128, 128], F32, tag="identf", name="identf")
    make_identity(nc, identf[:, :])
    identr = const.tile([128, 128], TRUNK, tag="identr", name="identr")
    nc.vector.tensor_scalar_add(out=identr[:, :], in0=identf[:, :], scalar1=0.0)
    identb = const.tile([128, 128], H16, tag="identb", name="identb")
    nc.vector.tensor_scalar_add(out=identb[:, :], in0=identf[:, :], scalar1=0.0)

    # sliding ones-column buffer: Z[:, 32] == 1 so Z[:, 32-r:64-r] has its
    # ones in column r; Z_slice.T @ U deposits column-sums of U into row r.
    zbuf = const.tile([128, 64], H16, tag="zbuf", name="zbuf")
    nc.vector.memset(zbuf[:, :], 0.0)
    nc.vector.memset(zbuf[:, 32:33], 1.0)

    # ---------------- weights: casting DMAs ----------------
    SPECIAL_KCH = {
        "cmp_W1": [(s * D + o, c) for s in range(4) for (o, c) in CH_D],
        "agg_W1": [(s * P + o, c) for s in range(4) for (o, c) in CH_P],
    }

    def load_w(name):
        h = io[name]
        K, M = h.shape
        dt = F32 if name in F32_WEIGHTS else H16
        tiles = []
        for i, (o, c) in enumerate(SPECIAL_KCH.get(name, _chunks(K))):
            t = wpool.tile([c, M], dt, tag=f"w_{name}_{i}", name=f"w_{name}_{i}")
            if dt == F32:
                nc.sync.dma_start(out=t[:, :], in_=h[o:o + c, :])
            else:
                stg = wstage.tile([128, M], F32, tag="wstg", name=f"wstg_{name}_{i}")
                nc.sync.dma_start(out=stg[:c, :], in_=h[o:o + c, :])
                nc.vector.tensor_scalar_add(out=t[:, :], in0=stg[:c, :], scalar1=0.0)
            tiles.append(t)
        return tiles

    def load_b(name):
        h = io[name]
        (M,) = h.shape
        tiles = []
        for i, (o, c) in enumerate(_chunks(M)):
            t = wpool.tile([c, 1], F32, tag=f"b_{name}_{i}", name=f"b_{name}_{i}")
            nc.sync.dma_start(out=t[:, :], in_=h[o:o + c])
            tiles.append(t)
        return tiles

    # ---------------- index DMAs + gathers (overlap weight DMAs) ----------
    pre2 = ctx.enter_context(ExitStack())
    g2pool = pre2.enter_context(tc.tile_pool(name="g2pool", bufs=1))
    pre1 = ctx.enter_context(ExitStack())
    gpool = pre1.enter_context(tc.tile_pool(name="gpool", bufs=1))
    e_n = {}
    for side, xh in (("1", io["x1"]), ("2", io["x2"])):
        for b in range(BL):
            idx = gpool.tile([128, 1], I32, tag=f"idx{side}_{b}", name=f"idx{side}_{b}")
            nc.sync.dma_start(out=idx[:, :], in_=xh[b, :])
            e = gpool.tile([128, D], H16, tag=f"e{side}_{b}", name=f"e{side}_{b}")
            nc.gpsimd.indirect_dma_start(
                out=e[:, :], out_offset=None, in_=io["emb"][:, :],
                in_offset=bass.IndirectOffsetOnAxis(ap=idx[:, :1], axis=0),
            )
            e_n[(side, b)] = e

    W = {}
    for n in WEIGHT_NAMES:
        W[n] = load_b(n) if n.endswith(("bh", "bt", "b1", "b2", "_b")) else load_w(n)

    # ---------------- helpers ----------------
    def mm_apply(w_tiles, b_tiles, rhs_tiles, n_free, func, out_tiles,
                 krange=None, mrange=None):
        """out = func(W.T @ rhs + b), transposed layout, 512-col PSUM chunks."""
        M = w_tiles[0].shape[1]
        mch = _chunks(M)
        ks = list(range(len(w_tiles))) if krange is None else krange
        m_iter = ([(i, i) for i in range(len(mch))] if mrange is None
                  else list(enumerate(mrange)))
        for oi, mi in m_iter:
            mo, mc = mch[mi]
            for fo in range(0, n_free, 512):
                fc = min(512, n_free - fo)
                ps = pp_mm.tile([128, 512], F32, tag="mmout", name="mmout")
                for idx, ki in enumerate(ks):
                    kc = w_tiles[ki].shape[0]
                    nc.tensor.matmul(
                        out=ps[:mc, :fc],
                        lhsT=w_tiles[ki][:kc, mo:mo + mc],
                        rhs=rhs_tiles[ki][:kc, fo:fo + fc],
                        start=(idx == 0),
                        stop=(idx == len(ks) - 1),
                    )
                nc.scalar.activation(
                    out=out_tiles[oi][:mc, fo:fo + fc],
                    in_=ps[:mc, :fc],
                    func=func, bias=b_tiles[mi][:mc, :], scale=1.0,
                )

    def highway(xt_tiles, wh, bh, wt, bt, feat, out_tiles):
        """out = x + t*(h-x), trunk layout, chunk-at-a-time (h reused as tmp)."""
        ch = _chunks(feat)
        for mi, (mo, mc) in enumerate(ch):
            h = work.tile([128, ROWS2], TRUNK, tag=f"hw_h{mi % 2}", name="hw_h")
            t = work.tile([128, ROWS2], TRUNK, tag=f"hw_t{mi % 2}", name="hw_t")
            mm_apply(wh, bh, xt_tiles, ROWS2, ACTF.Relu, [h], mrange=[mi])
            mm_apply(wt, bt, xt_tiles, ROWS2, ACTF.Sigmoid, [t], mrange=[mi])
            x_sl = xt_tiles[mi][:mc, :]
            nc.vector.tensor_tensor(out=h[:mc, :], in0=h[:mc, :], in1=x_sl,
                                    op=ALU.subtract)
            nc.vector.tensor_tensor(out=h[:mc, :], in0=h[:mc, :], in1=t[:mc, :],
                                    op=ALU.mult)
            nc.vector.tensor_tensor(out=out_tiles[mi][:mc, :], in0=h[:mc, :],
                                    in1=x_sl, op=ALU.add)

    # ---------------- embed: transpose into trunk ----------------
    eT = [g2pool.tile([128, ROWS2], TRUNK, tag=f"eT_{i}", name=f"eT_{i}")
          for i in range(3)]
    for ki, (ko, kc) in enumerate(CH_D):
        for side in ("1", "2"):
            ps = pp_tr.tile([128, 512], H16, tag="trpackb", name="trpack")
            for b in range(BL):
                nc.tensor.transpose(
                    out=ps[:kc, b * S:(b + 1) * S],
                    in_=e_n[(side, b)][:, ko:ko + kc],
                    identity=identb[:128, :128],
                )
            so = (0 if side == "1" else ROWS)
            nc.scalar.activation(out=eT[ki][:kc, so:so + ROWS], in_=ps[:kc, :ROWS],
                                 func=ACTF.Copy)
    pre1.close()  # frees index + gather tiles

    # ---------------- highway stack (trunk: both sides at once) -------------
    h1 = [g2pool.tile([128, ROWS2], TRUNK, tag=f"hwy1_{i}", name=f"hwy1_{i}")
          for i in range(3)]
    highway(eT, W["hw1_Wh"], W["hw1_bh"], W["hw1_Wt"], W["hw1_bt"], D, h1)
    eTh = [persist.tile([128, ROWS2], TRUNK, tag=f"eTh_{i}", name=f"eTh_{i}")
           for i in range(3)]
    highway(h1, W["hw2_Wh"], W["hw2_bh"], W["hw2_Wt"], W["hw2_bt"], D, eTh)
    pre2.close()  # frees eT, h1

    # ---------------- projections (shared weights, trunk) ----------------
    def proj(prefix, pool):
        z1 = [work.tile([128, ROWS2], TRUNK, tag=f"z1_{i}", name=f"z1_{i}") for i in range(2)]
        mm_apply(W[f"{prefix}_W1"], W[f"{prefix}_b1"], eTh, ROWS2, ACTF.Relu, z1)
        out = [pool.tile([128, ROWS2], TRUNK, tag=f"{prefix}T_{i}", name=f"{prefix}T_{i}")
               for i in range(2)]
        mm_apply(W[f"{prefix}_W2"], W[f"{prefix}_b2"], z1, ROWS2, ACTF.Relu, out)
        return out

    # dist first so the att2 elementwise can start while the PE continues
    # with the mul projection; hi chunk (m=1) first for earlier hi-prep
    qT = proj("dist", persist)
    # qT[0]: p 0..127 [128, (side,b,t)]; qT[1][:72]: p 128..199

    # ---- att2 prep: hi-chunk transposes to normal layout, per b ----
    # (pools opened here, after the gather/highway scratch is freed)
    q2ppool = ctx.enter_context(tc.tile_pool(name="q2ppool", bufs=8))
    uhipool = ctx.enter_context(tc.tile_pool(name="uhipool", bufs=2))
    q2rpool = ctx.enter_context(tc.tile_pool(name="q2rpool", bufs=2))
    ulopool = ctx.enter_context(tc.tile_pool(name="ulopool", bufs=3))
    shpool = ctx.enter_context(tc.tile_pool(name="shpool", bufs=2))
    q1n_hi, q2part, q2nh = {}, {}, {}
    for b in range(BL):
        ps = pp_tr.tile([128, 512], H16, tag="trpackb", name=f"hitr_{b}")
        nc.tensor.transpose(out=ps[:128, 0:PHI],
                            in_=qT[1][:PHI, b * S:(b + 1) * S],
                            identity=identr[:PHI, :PHI])
        nc.tensor.transpose(out=ps[:128, 128:128 + PHI],
                            in_=qT[1][:PHI, ROWS + b * S:ROWS + (b + 1) * S],
                            identity=identr[:PHI, :PHI])
        t1 = persist.tile([128, PHI], H16, tag=f"q1nh_{b}", name=f"q1nh_{b}")
        nc.vector.tensor_scalar_add(out=t1[:, :], in0=ps[:128, 0:PHI], scalar1=0.0)
        q1n_hi[b] = t1
        t2 = persist.tile([128, PHI], H16, tag=f"q2nh_{b}", name=f"q2nh_{b}")
        nc.vector.tensor_scalar_add(out=t2[:, :], in0=ps[:128, 128:128 + PHI],
                                    scalar1=0.0)
        q2nh[b] = t2

    # Q2PART[b,h]: [128 i, (64 j, p')], every partition = the flat q2n_hi
    # half-row; flatten seed + 7 doubling rounds. Emitted ROUND-MAJOR
    # across all 8 chains: each DMA's dependency was issued 8 slots
    # earlier, so the in-order sync queue never blocks on a waiting head.
    SHH = 64
    for b in range(BL):
        for h in range(2):
            q2part[(b, h)] = q2ppool.tile([128, SHH * PHI], H16, tag="q2part",
                                          name=f"q2p_{b}_{h}")
    for b in range(BL):
        for h in range(2):
            qp = q2part[(b, h)]
            qeng = (nc.sync, nc.sync, nc.gpsimd, nc.scalar)[b]
            qeng.dma_start(out=qp[0:1, :],
                           in_=q2nh[b][h * SHH:(h + 1) * SHH, :])
            n = 1
            while n < 128:
                qeng.dma_start(out=qp[n:2 * n, :], in_=qp[0:n, :])
                n *= 2

    # normal-layout post-highway embeddings (lhsT for the beta/alpha
    # matmuls); PE runs these during its att2 idle windows.
    ehw_n = {}
    for side in ("1", "2"):
        so = (0 if side == "1" else ROWS)
        for b in range(BL):
            ps = pp_tr.tile([128, 512], H16, tag="trpackb", name="trpackr")
            for ki, (ko, kc) in enumerate(CH_D):
                nc.tensor.transpose(
                    out=ps[:128, ko:ko + kc],
                    in_=eTh[ki][:kc, so + b * S:so + (b + 1) * S],
                    identity=identr[:kc, :kc],
                )
            t = persist.tile([128, D], H16, tag=f"ehwn{side}_{b}", name=f"ehwn{side}_{b}")
            nc.scalar.activation(out=t[:, :], in_=ps[:, :D], func=ACTF.Copy)
            ehw_n[(side, b)] = t

    def softmax_p(src_psum):
        """softmax over rows of src [128,128]; returns transposed probs fp16."""
        mx = small.tile([128, 1], F32, tag="sm_mx", name="sm_mx")
        nc.vector.tensor_reduce(out=mx[:, :], in_=src_psum, axis=AX.X,
                                op=ALU.max, negate=True)
        esb = small.tile([128, S], H16, tag="sm_e", name="sm_e")
        zs = small.tile([128, 1], F32, tag="sm_z", name="sm_z")
        nc.scalar.activation(out=esb[:, :], in_=src_psum, func=ACTF.Exp,
                             bias=mx[:, :], scale=1.0, accum_out=zs[:, :])
        rz = small.tile([128, 1], F32, tag="sm_rz", name="sm_rz")
        nc.vector.reciprocal(out=rz[:, :], in_=zs[:, :])
        pr = small.tile([128, S], H16, tag="sm_p", name="sm_p")
        nc.vector.tensor_scalar(out=pr[:, :], in0=esb[:, :], scalar1=rz[:, :],
                                scalar2=None, op0=ALU.mult)
        pt_ps = pp_tr.tile([128, 512], H16, tag="trpackb", name="trpackb")
        nc.tensor.transpose(out=pt_ps[:S, :S], in_=pr[:, :], identity=identb[:, :])
        pt = small.tile([128, S], H16, tag="sm_pt", name="sm_pt")
        nc.vector.tensor_scalar_add(out=pt[:, :], in0=pt_ps[:S, :S], scalar1=0.0)
        return pt

    betaT = {s: [persist.tile([128, 512], H16, tag=f"betaT{s}_{i}", name=f"betaT{s}_{i}")
                 for i in range(3)] for s in ("1", "2")}

    def sm_beta(b):
        bs4 = sim4[:, b * S:(b + 1) * S]
        simT_sb = small.tile([128, S], F32, tag="simT_sb", name="simT_sb")
        nc.vector.tensor_scalar_add(out=simT_sb[:, :], in0=bs4, scalar1=0.0)
        sim_ps = pp_sm.tile([128, S], F32, tag="btps", name="simtr")
        nc.tensor.transpose(out=sim_ps[:S, :S], in_=simT_sb[:, :],
                            identity=identf[:, :])
        ptA = softmax_p(bs4)  # alpha probs^T [i, j]
        ptB = softmax_p(sim_ps[:S, :S])  # beta probs^T [j, i]
        for side, pt, eln in (("2", ptA, "1"), ("1", ptB, "2")):
            for ki, (ko, kc) in enumerate(CH_D):
                bt_ps = pp_sm.tile([128, S], F32, tag="btps", name="btps")
                nc.tensor.matmul(
                    out=bt_ps[:kc, :], lhsT=ehw_n[(eln, b)][:, ko:ko + kc],
                    rhs=pt[:, :], start=True, stop=True,
                )
                nc.vector.tensor_scalar_add(
                    out=betaT[side][ki][:kc, b * S:(b + 1) * S],
                    in0=bt_ps[:kc, :], scalar1=0.0)

    # sim4 PSUM accumulates (in PE order): lo ones-matmuls + hi transposes
    # (emitted in the att2 loop, first one carries start=True) + att1
    # (emitted between windows once the mul projection is done).
    sim4 = pp_sim.tile([128, 512], F32, tag="sim4", name="sim4")
    sim4_group_started = [False] * 4

    # ---------------- att2 ----------------
    q1lo = qT[0][:PLO, 0:ROWS]  # [p, (b,i)]
    q2lo = qT[0][:PLO, ROWS:ROWS2]  # [p, (b,j)]

    W8 = 8  # replicated q2 stripe width for the LO subtract

    def raw_tt(out, in0, in1, op):
        """TensorTensor with shape-mismatched APs (same element order)."""
        eng = nc.vector
        return eng.add_instruction(
            mybir.InstTensorTensor(
                name=eng.bass.get_next_instruction_name(),
                op=op,
                ins=[eng.lower_ap(in0), eng.lower_ap(in1)],
                outs=[eng.lower_ap(out)],
            )
        )

    def lo_block(jb, abs_on_scalar=False):
        """LO p-chunk, j-block jb: u[p, (b, 8j, 128i)] -> ones-matmuls."""
        ncols = BL * JB * S  # 4096
        # q2r8: q2lo replicated along an i-stripe of width 8 (one 1x DVE
        # seed, 256 cols), laid out (b, j, i8). The subtract reads it with
        # a stride-0 MIDDLE dim (i-group): all inner dims contiguous -> 2x.
        q2r = q2rpool.tile([128, BL * JB * W8], H16, tag="q2r", name=f"q2r_{jb}")
        seed_in = bass.AP(
            tensor=q2lo.tensor, offset=q2lo.offset + jb * JB,
            ap=[q2lo.ap[0], [S, BL], [1, JB], [0, W8]])
        seed_out = bass.AP(
            tensor=q2r.tensor, offset=q2r.offset,
            ap=[q2r.ap[0], [JB * W8, BL], [W8, JB], [1, W8]])
        nc.vector.tensor_scalar_add(out=seed_out, in0=seed_in, scalar1=0.0)
        # subtract (TT 2x), ONE inst per block: both APs enumerate
        # (b, j, i) in the same order but with different dim structures
        # (bass's free-shape assert is bypassed; the ISA only streams).
        u = ulopool.tile([128, ncols], H16, tag="ulo", name=f"ulo_{jb}")
        in0 = bass.AP(tensor=q1lo.tensor, offset=q1lo.offset,
                      ap=[q1lo.ap[0], [S, BL], [0, JB], [1, S]])
        in1 = bass.AP(tensor=q2r.tensor, offset=q2r.offset,
                      ap=[q2r.ap[0], [W8, BL * JB], [0, S // W8], [1, W8]])
        uo = bass.AP(tensor=u.tensor, offset=u.offset,
                     ap=[u.ap[0], [1, ncols]])
        raw_tt(uo, in0, in1, ALU.subtract)
        if abs_on_scalar:
            # |u| then 1/(1+|u|), both on ScalarE (DVE relief)
            nc.scalar.activation(out=u[:, :], in_=u[:, :], func=ACTF.Abs)
        else:
            ui = u[:128, :].bitcast(I16)
            nc.vector.tensor_scalar(out=ui, in0=ui, scalar1=0x7FFF, scalar2=None,
                                    op0=ALU.bitwise_and)
        act_recip(nc, u[:, :], u[:, :], bias=1.0)
        # partition sums into sim4 rows via sliding ones-columns
        for jj in range(JB):
            j = jb * JB + jj
            g, rr = j // 32, j % 32
            rbase = u[:128, jj * S:jj * S + S]
            rhs = bass.AP(tensor=rbase.tensor, offset=rbase.offset,
                          ap=[rbase.ap[0], [JB * S, BL], [1, S]])
            nc.tensor.matmul(
                out=sim4[32 * g:32 * g + 32, :],
                lhsT=zbuf[:128, 32 - rr:64 - rr],
                rhs=rhs,
                start=(not sim4_group_started[g]), stop=False,
                skip_group_check=True,
                tile_position=(0, 32 * g),
            )
            sim4_group_started[g] = True

    SH = S // 2  # hi-path j-half size
    hi_done = []

    def hi_half(b, h):
        """HI p-chunk, batch b, j-half h: u[i, (64j, p')] -> tree -> transpose."""
        ncols = SH * PHI  # 4608
        qp = q2part[(b, h)]
        u = uhipool.tile([128, ncols], H16, tag="uhi", name=f"uhi_{b}_{h}")
        in0 = bass.AP(tensor=q1n_hi[b].tensor, offset=q1n_hi[b].offset,
                      ap=[q1n_hi[b].ap[0], [0, SH], [1, PHI]])
        u3 = u[:128, :].rearrange("i (j p) -> i j p", j=SH)
        qp3 = qp[:128, :].rearrange("i (j p) -> i j p", j=SH)
        nc.vector.tensor_tensor(out=u3, in0=in0, in1=qp3, op=ALU.subtract)
        ui = u[:128, :].bitcast(I16)
        nc.vector.tensor_scalar(out=ui, in0=ui, scalar1=0x7FFF, scalar2=None,
                                op0=ALU.bitwise_and)
        act_recip(nc, u[:, :], u[:, :], bias=1.0)
        # in-place fp16 tree folds: 72 -> 36 -> 18 (TT 2x)
        nc.vector.tensor_tensor(out=u3[:, :, 0:36], in0=u3[:, :, 0:36],
                                in1=u3[:, :, 36:72], op=ALU.add)
        nc.vector.tensor_tensor(out=u3[:, :, 0:18], in0=u3[:, :, 0:18],
                                in1=u3[:, :, 18:36], op=ALU.add)
        nc.vector.tensor_tensor(out=u3[:, :, 0:9], in0=u3[:, :, 0:9],
                                in1=u3[:, :, 9:18], op=ALU.add)
        sh = shpool.tile([128, SH], F32, tag="simhi", name=f"simhi_{b}_{h}")
        nc.vector.tensor_reduce(out=sh[:, :], in_=u3[:, :, 0:9], axis=AX.X,
                                op=ALU.add)
        shh = persist.tile([128, SH], H16, tag=f"simhih_{b}_{h}",
                           name=f"simhih_{b}_{h}")
        nc.vector.tensor_scalar_add(out=shh[:, :], in0=sh[:, :], scalar1=0.0)
        hi_done.append((b, h, shh))

    # interleave: att2 windows with the mul-projection pieces + att1 (PE
    # executes its queue in order, so trunk matmuls are spliced between
    # the windows' ones-matmuls).
    z1m = [work.tile([128, ROWS2], TRUNK, tag=f"z1_{i}", name=f"z1m_{i}")
           for i in range(2)]
    pT = [persist.tile([128, ROWS2], TRUNK, tag=f"mulT_{i}", name=f"mulT_{i}")
          for i in range(2)]

    def mul_piece(k):
        if k < 2:
            mm_apply(W["mul_W1"], W["mul_b1"], eTh, ROWS2, ACTF.Relu,
                     [z1m[k]], mrange=[k])
        else:
            mm_apply(W["mul_W2"], W["mul_b2"], z1m, ROWS2, ACTF.Relu,
                     [pT[k - 2]], mrange=[k - 2])

    ABS_SCALAR_BLOCKS = set()
    # all lo blocks first: they have no DMA dependency, so the in-order
    # DVE stream never stalls on a Q2PART chain while lo work remains
    for b in range(BL):
        for jb in range(4 * b, 4 * b + 4):
            lo_block(jb, abs_on_scalar=(jb in ABS_SCALAR_BLOCKS))
        mul_piece(b)
    # att1 (pT complete after the last mul piece)
    for bb in range(BL):
        for ki, (ko, kc) in enumerate(CH_P):
            nc.tensor.matmul(
                out=sim4[:, bb * S:(bb + 1) * S],
                lhsT=pT[ki][:kc, ROWS + bb * S:ROWS + (bb + 1) * S],
                rhs=pT[ki][:kc, bb * S:(bb + 1) * S],
                start=False, stop=False, skip_group_check=True,
            )
    # hi halves in chain-completion order; per b: transpose-accumulate
    # into sim4, then softmax+beta for that b immediately (its sim4
    # columns are complete) so the tail overlaps the next b's hi work
    for b in range(BL):
        for h in range(2):
            hi_half(b, h)
        for (bb, h, shh) in hi_done[-2:]:
            nc.tensor.matmul(
                out=sim4[h * SH:(h + 1) * SH, bb * S:(bb + 1) * S],
                lhsT=shh[:, :], rhs=identb[:, :],
                start=False, stop=False, skip_group_check=True,
                tile_position=(0, h * SH),
            )
        sm_beta(b)

    # close the sim4 accumulation group with a dummy stopping matmul on a
    # zero rhs? Not needed: readers below read PSUM after all writers.

    if debug and "dbg_sim4" in io:
        t = small.tile([128, 512], F32, tag="dbgps", name="dbg_sim4_t")
        nc.scalar.activation(out=t[:, :], in_=sim4[:, :], func=ACTF.Copy)
        nc.gpsimd.dma_start(out=io["dbg_sim4"][:, :], in_=t[:, :])

    # ---------------- softmax + beta/alpha + compare part 1 ----------------
    # cat + compare matmul, per side over 512-col trunk halves.
    cmp1 = [persist.tile([128, ROWS2], H16, tag=f"cmp1_{i}", name=f"cmp1_{i}")
            for i in range(2)]
    for side in ("1", "2"):
        so = (0 if side == "1" else ROWS)
        ps2 = [pp_mm.tile([128, 512], F32, tag="mmout", name=f"cmp1ps{mi}")
               for mi in range(2)]
        for sel in range(4):  # e, beta, e-beta, e*beta
            for ki, (ko, kc) in enumerate(CH_D):
                e_sl = eTh[ki][:kc, so:so + ROWS]
                b_sl = betaT[side][ki][:kc, :]
                if sel == 0:
                    rhs = e_sl  # pure-e chunk: use eTh directly, no copy
                elif sel == 1:
                    rhs = b_sl
                else:
                    cat = small.tile([128, 512], H16, tag="cat", name="cat")
                    nc.vector.tensor_tensor(
                        out=cat[:kc, :], in0=e_sl, in1=b_sl,
                        op=(ALU.subtract if sel == 2 else ALU.mult))
                    rhs = cat[:kc, :]
                idx = sel * 3 + ki
                for mi, (mo, mc) in enumerate(CH_P):
                    nc.tensor.matmul(
                        out=ps2[mi][:mc, :],
                        lhsT=W["cmp_W1"][idx][:, mo:mo + mc],
                        rhs=rhs,
                        start=(idx == 0), stop=(idx == 11),
                        skip_group_check=True,
                    )
        for mi, (mo, mc) in enumerate(CH_P):
            nc.scalar.activation(
                out=cmp1[mi][:mc, so:so + ROWS], in_=ps2[mi][:mc, :],
                func=ACTF.Relu,
                bias=W["cmp_b1"][mi][:mc, :], scale=1.0,
            )

    # ---------------- compare part 2 + compare highway (trunk) --------------
    v0 = [work.tile([128, ROWS2], H16, tag=f"z1_{i}", name=f"v0_{i}") for i in range(2)]
    mm_apply(W["cmp_W2"], W["cmp_b2"], cmp1, ROWS2, ACTF.Relu, v0)
    v1 = [work.tile([128, ROWS2], H16, tag=f"v1_{i}", name=f"v1_{i}") for i in range(2)]
    highway(v0, W["chw1_Wh"], W["chw1_bh"], W["chw1_Wt"], W["chw1_bt"], P, v1)
    vT = [persist.tile([128, ROWS2], H16, tag=f"vT_{i}", name=f"vT_{i}")
          for i in range(2)]
    highway(v1, W["chw2_Wh"], W["chw2_bh"], W["chw2_Wt"], W["chw2_bt"], P, vT)

    # ---------------- aggregate (fp32) ----------------
    stats = []
    for sect, (side, op) in enumerate(
            (("1", ALU.max), ("2", ALU.max), ("1", ALU.add), ("2", ALU.add))):
        so = (0 if side == "1" else ROWS)
        st = [persist.tile([128, BL], F32, tag=f"st{sect}_{i}", name=f"st{sect}_{i}")
              for i in range(2)]
        for ki, (ko, kc) in enumerate(CH_P):
            seg = vT[ki][:kc, so:so + ROWS].rearrange("p (b t) -> p b t", b=BL)
            nc.vector.tensor_reduce(
                out=st[ki][:kc, :BL], in_=seg, axis=AX.X, op=op,
            )
        stats.append(st)

    agg_rhs = [stats[s][ki] for s in range(4) for ki in range(2)]
    y1 = [persist.tile([128, BL], F32, tag=f"y1_{i}", name=f"y1_{i}") for i in range(2)]
    mm_apply(W["agg_W1"], W["agg_b1"], agg_rhs, BL, ACTF.Relu, y1)
    y2 = [persist.tile([128, BL], F32, tag=f"y2_{i}", name=f"y2_{i}") for i in range(2)]
    mm_apply(W["agg_W2"], W["agg_b2"], y1, BL, ACTF.Relu, y2)

    yt_ps = pp_sm.tile([128, S], F32, tag="btps", name="btps")
    for ki, (ko, kc) in enumerate(CH_P):
        nc.tensor.matmul(
            out=yt_ps[:C, :BL], lhsT=W["out_W"][ki][:kc, :],
            rhs=y2[ki][:kc, :], start=(ki == 0), stop=(ki == 1),
        )
    yt_sb = persist.tile([C, BL], F32, tag="yt_sb", name="yt_sb")
    nc.scalar.activation(out=yt_sb[:, :], in_=yt_ps[:C, :BL], func=ACTF.Identity,
                         bias=W["out_b"][0][:C, :], scale=1.0)
    nc.sync.dma_start(out=io["yt"][:, :], in_=yt_sb[:, :])


_NC_CACHE = {}


def _get_nc():
    if "nc" not in _NC_CACHE:
        _NC_CACHE["nc"] = build_nc()
    return _NC_CACHE["nc"]


def make_in_maps(inputs):
    """Shard full inputs into 8 per-core input maps."""
    x1 = np.ascontiguousarray(np.asarray(inputs["x1"]).astype(np.int32))
    x2 = np.ascontiguousarray(np.asarray(inputs["x2"]).astype(np.int32))
    shared = {}
    for n in WEIGHT_NAMES + ["emb"]:
        shared[n] = np.ascontiguousarray(np.asarray(inputs[n]).astype(np.float32))
    in_maps = []
    for c in range(NCORES):
        m = dict(shared)
        m["x1"] = x1[c * BL:(c + 1) * BL]
        m["x2"] = x2[c * BL:(c + 1) * BL]
        in_maps.append(m)
    return in_maps


def kernel(**inputs):
    nc = _get_nc()
    in_maps = make_in_maps(inputs)
    res = run_bass_kernel_spmd(nc, in_maps, core_ids=list(range(NCORES)))
    return np.concatenate([np.asarray(r["yt"]).T for r in res.results], axis=0)


if __name__ == "__main__":
    nc = build_nc()
    print("built ok")
